# revision 34
# baseline (speedup 1.0000x reference)
"""GQA (B=2,S=1024,E=4096,H=32,KV=8,HD=128, RoPE, no causal mask) on 8 NeuronCores.

Sharding: 2 batch-groups x 4-way head tensor-parallel.
Core c: batch b=c//4, tp rank r=c%4 -> 8 q heads [8r,8r+8), 2 kv heads [2r,2r+2),
wo rows [1024r, 1024(r+1)).  Each core computes a partial output
out_part = y_local @ wo[local_rows, :]  (emitted transposed as [4096, 1024] fp16);
host sums the 4 partials per batch. No device collectives needed.

v4: single fused pipeline.
- Projections are chunk-major (full-E accumulation in PSUM), order
  K0,K1,V0 interleaved per e-chunk (tracks the x DMA stream), V1, Q0..Q7.
- Head h's QK+exp / PV / y-transpose are slot-scheduled into chunk h+1 / h+2's
  projection groups so scalar-engine exp (~110us) and all DVE chains hide
  under Tensor work.
- Weights are host-prearranged so every weight DMA is contiguous per
  partition; output DMA is fp16 (host accumulates partials in fp32).
"""
import sys

sys.path.insert(0, "/opt/trn_rl_repo")

import numpy as np

B = 2
S = 1024
E = 4096
HD = 128
N_CORES = 8
TP = 4            # tensor-parallel ranks per batch group
HL = 8            # q heads per core
KVL = 2           # kv heads per core
QCOLS = HL * HD   # 1024
KVCOLS = KVL * HD  # 256
ECH = E // 128    # 32 e-chunks
TT = S // 128     # 8 token tiles
SCALE = 1.0 / np.sqrt(np.float32(HD))
MM_DT = "float16"

_PROGRAM = None


def _build_program():
    import concourse.bass as bass  # noqa: F401
    from concourse import bacc
    import concourse.mybir as mybir
    from concourse.tile import TileContext
    from concourse.masks import make_identity

    dt = mybir.dt.float32
    dtr = getattr(mybir.dt, MM_DT)
    nc = bacc.Bacc("TRN2", target_bir_lowering=False, debug=False,
                   num_devices=N_CORES)

    xt_d = nc.declare_dram_parameter("xt", [E, S], dtr, isOutput=False)
    # host-prearranged: row block cc*128+p holds w[:, cc*128:...] row c*128+p
    wq_d = nc.declare_dram_parameter("wq", [HL * 128, E], dtr, isOutput=False)
    wk_d = nc.declare_dram_parameter("wk", [KVL * 128, E], dtr, isOutput=False)
    wv_d = nc.declare_dram_parameter("wv", [KVL * 128, E], dtr, isOutput=False)
    wo_d = nc.declare_dram_parameter("wo", [ECH * 128, QCOLS], dtr,
                                     isOutput=False)
    cos_d = nc.declare_dram_parameter("cos", [HD, S], dtr, isOutput=False)
    sinp_d = nc.declare_dram_parameter("sinp", [HD, S], dtr, isOutput=False)
    out_d = nc.declare_dram_parameter("out_t", [E, S], dtr, isOutput=True)

    def w_src(cc):
        # [128, ECH, 128] view of chunk cc's weights, contiguous per partition
        if cc < HL:
            base = wq_d
        elif cc < HL + KVL:
            base, cc = wk_d, cc - HL
        else:
            base, cc = wv_d, cc - HL - KVL
        return base[cc * 128:(cc + 1) * 128, :].rearrange(
            "p (c m) -> p c m", m=128)

    with TileContext(nc) as tc:
        with tc.tile_pool(name="const", bufs=1) as cpool, \
             tc.tile_pool(name="persist", bufs=1) as ppool, \
             tc.tile_pool(name="vnat", bufs=1) as vpool, \
             tc.tile_pool(name="wstream", bufs=5) as wpool, \
             tc.tile_pool(name="qroll", bufs=3) as qpool, \
             tc.tile_pool(name="rope", bufs=2) as ropool:
            ident_f = cpool.tile([128, 128], dt)
            make_identity(nc, ident_f[:])
            ident = cpool.tile([128, 128], dtr)
            nc.scalar.copy(ident[:], ident_f[:])
            cos_t = cpool.tile([HD, S], dtr, tag="cos")
            sinp_t = cpool.tile([HD, S], dtr, tag="sinp")

            # persistent data
            xs = ppool.tile([128, ECH, S], dtr, tag="xs", name="xs")
            kT = [ppool.tile([128, S], dtr, tag=f"kT{i}", name=f"kT{i}")
                  for i in range(KVL)]
            yT = [ppool.tile([128, S], dtr, tag=f"yT{i}", name=f"yT{i}")
                  for i in range(HL)]
            v_nat = [[vpool.tile([128, HD + 1], dtr, tag=f"v{kv}_{kt}",
                                 name=f"v{kv}_{kt}")
                      for kt in range(TT)] for kv in range(KVL)]

            # DMA emission order matters: the Sync engine issues descriptors
            # in order at ~240-330GB/s aggregate, so stage the first four
            # chunks' weights per-superchunk between x slices.
            pre_cc = [HL, HL + 1, HL + KVL, HL + KVL + 1]  # K0, K1, V0, V1
            pre_wt = [wpool.tile([128, ECH, 128], dtr, tag="w",
                                 name=f"wt_pre{j}") for j in range(4)]
            for es in range(4):
                wjs = [0, 1, 2, 3] if es else [0]
                if es == 0:  # first matmul needs wt_k0[es0] + xs[0] first
                    nc.sync.dma_start(out=pre_wt[0][:, 0:8, :],
                                      in_=w_src(pre_cc[0])[:, 0:8, :])
                    nc.sync.dma_start(out=xs[:, 0, :], in_=xt_d[0:128, :])
                    wjs = [1, 2, 3]
                for j in wjs:
                    nc.sync.dma_start(
                        out=pre_wt[j][:, es * 8:(es + 1) * 8, :],
                        in_=w_src(pre_cc[j])[:, es * 8:(es + 1) * 8, :])
                for ec in range(es * 8 + (1 if es == 0 else 0), (es + 1) * 8):
                    nc.sync.dma_start(out=xs[:, ec, :],
                                      in_=xt_d[ec * 128:(ec + 1) * 128, :])
                if es == 1:
                    nc.sync.dma_start(out=cos_t[:], in_=cos_d[:])
                    nc.sync.dma_start(out=sinp_t[:], in_=sinp_d[:])

            def w_dma(cc, name):
                wt = wpool.tile([128, ECH, 128], dtr, tag="w", name=name)
                nc.sync.dma_start(out=wt[:], in_=w_src(cc)[:])
                return wt

            def rope_half(dstT, acc, tb):
                lo, hi = tb * 512, (tb + 1) * 512
                tmp = ropool.tile([HD, 512], dtr, tag=f"t0{tb}", name="tmp")
                nc.scalar.copy(tmp[:], acc[:, lo:hi])
                sh = ropool.tile([HD, 512], dtr, tag=f"sh{tb}", name="sh")
                nc.sync.dma_start(out=sh[0:64, :], in_=tmp[64:128, :])
                nc.sync.dma_start(out=sh[64:128, :], in_=tmp[0:64, :])
                t1 = ropool.tile([HD, 512], dtr, tag=f"t1{tb}", name="t1")
                nc.vector.tensor_mul(t1[:], tmp[:], cos_t[:, lo:hi])
                nc.vector.tensor_mul(sh[:], sh[:], sinp_t[:, lo:hi])
                nc.vector.tensor_add(dstT[:, lo:hi], t1[:], sh[:])

            # ---------------- pre-head phase ----------------
            # all four K/V chunks interleaved per e-chunk so compute tracks
            # the x DMA stream (4 accumulators = all 8 PSUM banks)
            vtmps = []
            qT = [None] * HL
            with tc.tile_pool(name="psPre", bufs=1, space="PSUM") as psPre:
                pre_acc = [psPre.tile([128, S], dt, tag="acc", bufs=4,
                                      name=f"accp{j}") for j in range(4)]
                for ec in range(ECH):
                    for j in range(4):
                        for tb in range(2):
                            nc.tensor.matmul(
                                pre_acc[j][:, tb * 512:(tb + 1) * 512],
                                pre_wt[j][:, ec, :],
                                xs[:, ec, tb * 512:(tb + 1) * 512],
                                start=(ec == 0), stop=(ec == ECH - 1),
                                skip_group_check=True)
                # chunk Q0 accumulates in psPre's slot ring (overlays the
                # earliest-freed accumulator) so the PE never waits for the
                # late-freed V accumulators' address reuse
                wt_q0 = w_dma(0, "wt_q0")
                acc_q0 = psPre.tile([128, S], dt, tag="acc", bufs=4,
                                    name="acc_q0")
                qT[0] = qpool.tile([128, S], dtr, tag="qT", name="qT0")
                for tb in range(2):
                    for ec in range(ECH):
                        nc.tensor.matmul(
                            acc_q0[:, tb * 512:(tb + 1) * 512],
                            wt_q0[:, ec, :],
                            xs[:, ec, tb * 512:(tb + 1) * 512],
                            start=(ec == 0), stop=(ec == ECH - 1),
                            skip_group_check=True)
                    rope_half(qT[0], acc_q0, tb)
                for i in range(KVL):
                    rope_half(kT[i], pre_acc[i], 0)
                    rope_half(kT[i], pre_acc[i], 1)
                for i in range(KVL):
                    vtmp = ropool.tile([128, S], dtr, tag=f"vt{i}",
                                       name="vtmp", bufs=1)
                    nc.vector.tensor_copy(vtmp[:, 0:512],
                                          pre_acc[2 + i][:, 0:512])
                    nc.vector.tensor_copy(vtmp[:, 512:S],
                                          pre_acc[2 + i][:, 512:S])
                    vtmps.append(vtmp)

            # ---------------- head loop ----------------
            # slot schedule inside chunk `it`'s projection, group g (0..7):
            #   g=0:  fin(it-3, 6), pv(it-3, 7), qk(it-1, 0)
            #   g=1:  fin(it-3, 7), pv(it-2, 0), qk(it-1, 1)
            #   g>=2: fin(it-2, g-2), pv(it-2, g-1), qk(it-1, g)
            # so every transpose (fin) trails its PV block by two groups and
            # exp for head it-1 is paced across the whole chunk.
            from contextlib import ExitStack
            with tc.tile_pool(name="psSmall", bufs=4, space="PSUM") as psSm, \
                 tc.tile_pool(name="pt", bufs=18) as ptpool, \
                 tc.tile_pool(name="ynorm", bufs=3) as ypool, \
                 tc.tile_pool(name="recs", bufs=3) as recpool, \
                 tc.tile_pool(name="wo", bufs=3) as wopool, \
                 tc.tile_pool(name="osb", bufs=3) as opool:
                qk_stack = ExitStack()
                psProj = qk_stack.enter_context(
                    tc.tile_pool(name="psProj", bufs=2, space="PSUM"))
                pts = [[None] * TT for _ in range(HL)]
                ysbs = {}

                def qk_pair(h, kc):
                    kv = h // (HL // KVL)
                    pts[h][kc] = ptpool.tile([128, S], dtr, tag="pt",
                                             name=f"pt{h}_{kc}")
                    for tb in range(2):
                        sp = psSm.tile([128, 512], dt, tag="small", name="sp")
                        nc.tensor.matmul(
                            sp[:], kT[kv][:, kc * 128:(kc + 1) * 128],
                            qT[h][:, tb * 512:(tb + 1) * 512],
                            start=True, stop=True, skip_group_check=True)
                        nc.scalar.activation(
                            pts[h][kc][:, tb * 512:(tb + 1) * 512], sp[:],
                            mybir.ActivationFunctionType.Exp,
                            scale=float(SCALE))

                def pv_mm(h, qt):
                    kv = h // (HL // KVL)
                    yp = psSm.tile([128, 512], dt, tag="small", name="yp")
                    for kc in range(TT):
                        nc.tensor.matmul(
                            yp[:, 0:HD + 1],
                            pts[h][kc][:, qt * 128:(qt + 1) * 128],
                            v_nat[kv][kc][:],
                            start=(kc == 0), stop=(kc == TT - 1),
                            skip_group_check=True)
                    rec = recpool.tile([128, 1], dt, tag="rec", name="rec")
                    nc.vector.reciprocal(rec[:], yp[:, HD:HD + 1])
                    ysb = ypool.tile([128, HD], dtr, tag="ysb", name="ysb")
                    nc.vector.tensor_scalar_mul(ysb[:], yp[:, 0:HD], rec[:])
                    ysbs[(h, qt)] = ysb

                def pv_fin(h, qt):
                    ysb = ysbs.pop((h, qt))
                    ytp = psSm.tile([128, 128], dtr, tag="small", name="ytp")
                    nc.tensor.transpose(ytp[:], ysb[:], ident[:])
                    nc.vector.tensor_copy(yT[h][:, qt * 128:(qt + 1) * 128],
                                          ytp[:])

                extra_q = []

                def head_step(it, g, extra=None):
                    if g == 0:
                        fin_h, fin_qt = it - 3, 6
                        pv_h, pv_qt = it - 3, 7
                    elif g == 1:
                        fin_h, fin_qt = it - 3, 7
                        pv_h, pv_qt = it - 2, 0
                    else:
                        fin_h, fin_qt = it - 2, g - 2
                        pv_h, pv_qt = it - 2, g - 1
                    # qk/pv matmuls run before each fin transpose so the DVE
                    # normalize chain it depends on is always covered
                    if g > 0 and 0 <= it - 1 < HL and g < TT:
                        qk_pair(it - 1, g)
                    if extra is not None:
                        extra()
                    for _ in range(2):
                        if extra_q:
                            extra_q.pop(0)()
                    if 0 <= pv_h < HL and pv_qt < TT:
                        pv_mm(pv_h, pv_qt)
                    if 0 <= fin_h < HL and (fin_h, fin_qt) in ysbs:
                        pv_fin(fin_h, fin_qt)
                    if g == 0 and 0 <= it - 1 < HL:
                        qk_pair(it - 1, 0)

                def head_chunk(it, wt):
                    # tb-outer so the first half's rope overlaps the second
                    # half's matmuls, shortening the qT critical chain
                    acc = psProj.tile([128, S], dt, tag="acc", bufs=2,
                                      name="acc")
                    q = qpool.tile([128, S], dtr, tag="qT", name=f"qT{it}")
                    n = 0
                    for tb in range(2):
                        for ec in range(ECH):
                            nc.tensor.matmul(
                                acc[:, tb * 512:(tb + 1) * 512], wt[:, ec, :],
                                xs[:, ec, tb * 512:(tb + 1) * 512],
                                start=(ec == 0), stop=(ec == ECH - 1),
                                skip_group_check=True)
                            n += 1
                            if it >= 1 and n % 8 == 0:
                                head_step(it, n // 8 - 1)
                        rope_half(q, acc, tb)
                    return q

                def v_unit(i, kt):
                    pt = psSm.tile([128, 128], dtr, tag="small", name="vtp")
                    nc.tensor.transpose(
                        pt[:], vtmps[i][:, kt * 128:(kt + 1) * 128], ident[:])
                    nc.vector.tensor_copy(v_nat[i][kt][:, 0:HD], pt[:])
                    nc.vector.memset(v_nat[i][kt][:, HD:HD + 1], 1.0)

                # V transposes slot into chunk Q1's interleave groups
                extra_q.extend(
                    lambda i=i, kt=kt: v_unit(i, kt)
                    for i in range(KVL) for kt in range(TT))
                for it in range(1, HL):
                    wt = w_dma(it, f"wt_q{it}")
                    qT[it] = head_chunk(it, wt)
                # virtual iteration 8 drains QK of head 7 + PV of heads 5/6
                for g in range(TT):
                    head_step(HL, g)
                qk_stack.close()  # free psProj banks for psO

                def wo_dma(oc):
                    wt = wopool.tile([128, HL, 128], dtr, tag="wo",
                                     name=f"wt_o{oc}")
                    nc.sync.dma_start(
                        out=wt[:],
                        in_=wo_d[oc * 128:(oc + 1) * 128, :].rearrange(
                            "p (c m) -> p c m", m=128))
                    return wt

                def e_half(op, wt, oc, tb, yc_list, start, stop, ot=None):
                    for yc in yc_list:
                        nc.tensor.matmul(
                            op[:, tb * 512:(tb + 1) * 512], wt[:, yc, :],
                            yT[yc][:, tb * 512:(tb + 1) * 512],
                            start=(start and yc == yc_list[0]),
                            stop=(stop and yc == yc_list[-1]),
                            skip_group_check=True)
                    if ot is not None:
                        nc.scalar.copy(ot[:, tb * 512:(tb + 1) * 512],
                                       op[:, tb * 512:(tb + 1) * 512])
                        nc.sync.dma_start(
                            out=out_d[oc * 128:(oc + 1) * 128,
                                      tb * 512:(tb + 1) * 512],
                            in_=ot[:, tb * 512:(tb + 1) * 512])

                # ------------ out projection (partial, transposed, fp16) ----
                # oc 0/1 accumulate heads 0-5 interleaved into the PV drain of
                # heads 6/7, so the tail never idles the PE
                with tc.tile_pool(name="psO", bufs=2, space="PSUM") as psO:
                    wt_o01 = [wo_dma(0), wo_dma(1)]
                    op01 = [psO.tile([128, S], dt, tag="op", name=f"op{j}")
                            for j in range(2)]
                    ethunks = []
                    for j in range(2):
                        for tb in range(2):
                            for y0 in (0, 2, 4):
                                ethunks.append(
                                    lambda j=j, tb=tb, y0=y0: e_half(
                                        op01[j], wt_o01[j], j, tb,
                                        [y0, y0 + 1], start=(y0 == 0),
                                        stop=False))
                    # yc=6 contributions become legal once head 6 finishes
                    # (virtual iteration 9, group 1) — keep them last
                    for j in range(2):
                        for tb in range(2):
                            ethunks.append(
                                lambda j=j, tb=tb: e_half(
                                    op01[j], wt_o01[j], j, tb, [6],
                                    start=False, stop=False))

                    def extra2():
                        for _ in range(2):
                            if ethunks:
                                ethunks.pop(0)()

                    for g in range(TT):
                        head_step(HL + 1, g, extra=extra2)
                    head_step(HL + 2, 0, extra=extra2)
                    head_step(HL + 2, 1, extra=extra2)
                    while ethunks:
                        ethunks.pop(0)()
                    for j in range(2):
                        ot = opool.tile([128, S], dtr, tag="ot", name="ot")
                        for tb in range(2):
                            e_half(op01[j], wt_o01[j], j, tb, [7],
                                   start=False, stop=True, ot=ot)
                    for oc in range(2, E // 128):
                        wt = wo_dma(oc)
                        op = psO.tile([128, S], dt, tag="op", name="op")
                        ot = opool.tile([128, S], dtr, tag="ot", name="ot")
                        for tb in range(2):
                            e_half(op, wt, oc, tb, list(range(HL)),
                                   start=True, stop=True, ot=ot)

    nc.compile()
    return nc


def _rope_tables():
    inv = 1.0 / (10000.0 ** (np.arange(0, HD, 2, dtype=np.float32) / HD))  # [64]
    ang = np.arange(S, dtype=np.float32)[None, :] * inv[:, None]           # [64, S]
    cos = np.concatenate([np.cos(ang), np.cos(ang)], axis=0).astype(np.float32)   # [128, S]
    sin = np.sin(ang)
    sinp = np.concatenate([-sin, sin], axis=0).astype(np.float32)          # [128, S]
    return cos, sinp


def _rearrange_w(w, n_chunks):
    # [E_rows, n_chunks*128] -> [n_chunks*128, E_rows] blocks: row cc*128+p
    # holds w[c*128+p, cc*128+m] at col c*128+m
    e_rows = w.shape[0]
    c = e_rows // 128
    return np.ascontiguousarray(
        w.reshape(c, 128, n_chunks, 128).transpose(2, 1, 0, 3).reshape(
            n_chunks * 128, e_rows))


def make_in_maps(x, wq, wk, wv, wo):
    cos, sinp = _rope_tables()
    ndt = np.float16 if MM_DT == "float16" else np.float32
    x = np.ascontiguousarray(x, dtype=np.float32)
    in_maps = []
    for c in range(N_CORES):
        b, r = c // TP, c % TP
        in_maps.append({
            "xt": np.ascontiguousarray(x[b].T).astype(ndt),
            "wq": _rearrange_w(
                wq[:, r * QCOLS:(r + 1) * QCOLS].astype(ndt), HL),
            "wk": _rearrange_w(
                wk[:, r * KVCOLS:(r + 1) * KVCOLS].astype(ndt), KVL),
            "wv": _rearrange_w(
                wv[:, r * KVCOLS:(r + 1) * KVCOLS].astype(ndt), KVL),
            "wo": _rearrange_w(
                wo[r * QCOLS:(r + 1) * QCOLS, :].astype(ndt), ECH),
            "cos": cos.astype(ndt),
            "sinp": sinp.astype(ndt),
        })
    return in_maps


def kernel(x, wq, wk, wv, wo):
    global _PROGRAM
    from concourse.bass_utils import run_bass_kernel_spmd

    if _PROGRAM is None:
        _PROGRAM = _build_program()
    nc = _PROGRAM

    res = run_bass_kernel_spmd(nc, make_in_maps(x, wq, wk, wv, wo),
                               list(range(N_CORES)))

    out = np.zeros((B, S, E), dtype=np.float32)
    for c in range(N_CORES):
        b = c // TP
        out[b] += res.results[c]["out_t"].T.astype(np.float32)
    return out


# revision 35
# speedup vs baseline: 1.0040x; 1.0040x over previous
"""GQA (B=2,S=1024,E=4096,H=32,KV=8,HD=128, RoPE, no causal mask) on 8 NeuronCores.

Sharding: 2 batch-groups x 4-way head tensor-parallel.
Core c: batch b=c//4, tp rank r=c%4 -> 8 q heads [8r,8r+8), 2 kv heads [2r,2r+2),
wo rows [1024r, 1024(r+1)).  Each core computes a partial output
out_part = y_local @ wo[local_rows, :]  (emitted transposed as [4096, 1024] fp16);
host sums the 4 partials per batch. No device collectives needed.

v4: single fused pipeline.
- Projections are chunk-major (full-E accumulation in PSUM), order
  K0,K1,V0 interleaved per e-chunk (tracks the x DMA stream), V1, Q0..Q7.
- Head h's QK+exp / PV / y-transpose are slot-scheduled into chunk h+1 / h+2's
  projection groups so scalar-engine exp (~110us) and all DVE chains hide
  under Tensor work.
- Weights are host-prearranged so every weight DMA is contiguous per
  partition; output DMA is fp16 (host accumulates partials in fp32).
"""
import sys

sys.path.insert(0, "/opt/trn_rl_repo")

import numpy as np

B = 2
S = 1024
E = 4096
HD = 128
N_CORES = 8
TP = 4            # tensor-parallel ranks per batch group
HL = 8            # q heads per core
KVL = 2           # kv heads per core
QCOLS = HL * HD   # 1024
KVCOLS = KVL * HD  # 256
ECH = E // 128    # 32 e-chunks
TT = S // 128     # 8 token tiles
SCALE = 1.0 / np.sqrt(np.float32(HD))
MM_DT = "float16"

_PROGRAM = None


def _build_program():
    import concourse.bass as bass  # noqa: F401
    from concourse import bacc
    import concourse.mybir as mybir
    from concourse.tile import TileContext
    from concourse.masks import make_identity

    dt = mybir.dt.float32
    dtr = getattr(mybir.dt, MM_DT)
    nc = bacc.Bacc("TRN2", target_bir_lowering=False, debug=False,
                   num_devices=N_CORES)

    xt_d = nc.declare_dram_parameter("xt", [E, S], dtr, isOutput=False)
    # host-prearranged: row block cc*128+p holds w[:, cc*128:...] row c*128+p
    wq_d = nc.declare_dram_parameter("wq", [HL * 128, E], dtr, isOutput=False)
    wk_d = nc.declare_dram_parameter("wk", [KVL * 128, E], dtr, isOutput=False)
    wv_d = nc.declare_dram_parameter("wv", [KVL * 128, E], dtr, isOutput=False)
    wo_d = nc.declare_dram_parameter("wo", [ECH * 128, QCOLS], dtr,
                                     isOutput=False)
    cos_d = nc.declare_dram_parameter("cos", [HD, S], dtr, isOutput=False)
    sinp_d = nc.declare_dram_parameter("sinp", [HD, S], dtr, isOutput=False)
    out_d = nc.declare_dram_parameter("out_t", [E, S], dtr, isOutput=True)

    def w_src(cc):
        # [128, ECH, 128] view of chunk cc's weights, contiguous per partition
        if cc < HL:
            base = wq_d
        elif cc < HL + KVL:
            base, cc = wk_d, cc - HL
        else:
            base, cc = wv_d, cc - HL - KVL
        return base[cc * 128:(cc + 1) * 128, :].rearrange(
            "p (c m) -> p c m", m=128)

    with TileContext(nc) as tc:
        with tc.tile_pool(name="const", bufs=1) as cpool, \
             tc.tile_pool(name="persist", bufs=1) as ppool, \
             tc.tile_pool(name="vnat", bufs=1) as vpool, \
             tc.tile_pool(name="wstream", bufs=5) as wpool, \
             tc.tile_pool(name="qroll", bufs=3) as qpool, \
             tc.tile_pool(name="rope", bufs=2) as ropool:
            ident_f = cpool.tile([128, 128], dt)
            make_identity(nc, ident_f[:])
            ident = cpool.tile([128, 128], dtr)
            nc.scalar.copy(ident[:], ident_f[:])
            cos_t = cpool.tile([HD, S], dtr, tag="cos")
            sinp_t = cpool.tile([HD, S], dtr, tag="sinp")

            # persistent data
            xs = ppool.tile([128, ECH, S], dtr, tag="xs", name="xs")
            kT = [ppool.tile([128, S], dtr, tag=f"kT{i}", name=f"kT{i}")
                  for i in range(KVL)]
            yT = [ppool.tile([128, S], dtr, tag=f"yT{i}", name=f"yT{i}")
                  for i in range(HL)]
            v_nat = [[vpool.tile([128, HD + 1], dtr, tag=f"v{kv}_{kt}",
                                 name=f"v{kv}_{kt}")
                      for kt in range(TT)] for kv in range(KVL)]

            # DMA emission order matters: the Sync engine issues descriptors
            # in order at ~240-330GB/s aggregate, so stage the first four
            # chunks' weights per-superchunk between x slices.
            pre_cc = [HL, HL + 1, HL + KVL, HL + KVL + 1]  # K0, K1, V0, V1
            pre_wt = [wpool.tile([128, ECH, 128], dtr, tag="w",
                                 name=f"wt_pre{j}") for j in range(4)]
            for es in range(4):
                wjs = [0, 1, 2, 3] if es else [0]
                if es == 0:  # first matmul needs wt_k0[es0] + xs[0] first
                    nc.sync.dma_start(out=pre_wt[0][:, 0:8, :],
                                      in_=w_src(pre_cc[0])[:, 0:8, :])
                    nc.sync.dma_start(out=xs[:, 0, :], in_=xt_d[0:128, :])
                    wjs = [1, 2, 3]
                for j in wjs:
                    nc.sync.dma_start(
                        out=pre_wt[j][:, es * 8:(es + 1) * 8, :],
                        in_=w_src(pre_cc[j])[:, es * 8:(es + 1) * 8, :])
                for ec in range(es * 8 + (1 if es == 0 else 0), (es + 1) * 8):
                    nc.sync.dma_start(out=xs[:, ec, :],
                                      in_=xt_d[ec * 128:(ec + 1) * 128, :])
                if es == 1:
                    nc.sync.dma_start(out=cos_t[:], in_=cos_d[:])
                    nc.sync.dma_start(out=sinp_t[:], in_=sinp_d[:])

            def w_dma(cc, name):
                wt = wpool.tile([128, ECH, 128], dtr, tag="w", name=name)
                nc.sync.dma_start(out=wt[:], in_=w_src(cc)[:])
                return wt

            def rope_half(dstT, acc, tb):
                lo, hi = tb * 512, (tb + 1) * 512
                tmp = ropool.tile([HD, 512], dtr, tag=f"t0{tb}", name="tmp")
                nc.scalar.copy(tmp[:], acc[:, lo:hi])
                sh = ropool.tile([HD, 512], dtr, tag=f"sh{tb}", name="sh")
                nc.sync.dma_start(out=sh[0:64, :], in_=tmp[64:128, :])
                nc.sync.dma_start(out=sh[64:128, :], in_=tmp[0:64, :])
                t1 = ropool.tile([HD, 512], dtr, tag=f"t1{tb}", name="t1")
                nc.vector.tensor_mul(t1[:], tmp[:], cos_t[:, lo:hi])
                nc.vector.tensor_mul(sh[:], sh[:], sinp_t[:, lo:hi])
                nc.vector.tensor_add(dstT[:, lo:hi], t1[:], sh[:])

            # ---------------- pre-head phase ----------------
            # all four K/V chunks interleaved per e-chunk so compute tracks
            # the x DMA stream (4 accumulators = all 8 PSUM banks)
            vtmps = []
            qT = [None] * HL
            with tc.tile_pool(name="psPre", bufs=1, space="PSUM") as psPre:
                pre_acc = [psPre.tile([128, S], dt, tag="acc", bufs=4,
                                      name=f"accp{j}") for j in range(4)]
                for ec in range(ECH):
                    for j in range(4):
                        for tb in range(2):
                            nc.tensor.matmul(
                                pre_acc[j][:, tb * 512:(tb + 1) * 512],
                                pre_wt[j][:, ec, :],
                                xs[:, ec, tb * 512:(tb + 1) * 512],
                                start=(ec == 0), stop=(ec == ECH - 1),
                                skip_group_check=True)
                # chunk Q0 accumulates in psPre's slot ring (overlays the
                # earliest-freed accumulator) so the PE never waits for the
                # late-freed V accumulators' address reuse
                wt_q0 = w_dma(0, "wt_q0")
                acc_q0 = psPre.tile([128, S], dt, tag="acc", bufs=4,
                                    name="acc_q0")
                qT[0] = qpool.tile([128, S], dtr, tag="qT", name="qT0")
                wts = [None] * (HL + 1)
                wts[0] = wt_q0
                for tb in range(2):
                    for ec in range(ECH):
                        nc.tensor.matmul(
                            acc_q0[:, tb * 512:(tb + 1) * 512],
                            wt_q0[:, ec, :],
                            xs[:, ec, tb * 512:(tb + 1) * 512],
                            start=(ec == 0), stop=(ec == ECH - 1),
                            skip_group_check=True)
                    if tb == 0:
                        # next chunk's weights issue before any rope swap
                        # DMAs enter the in-order Sync queue
                        wts[1] = w_dma(1, "wt_q1")
                    rope_half(qT[0], acc_q0, tb)
                for i in range(KVL):
                    rope_half(kT[i], pre_acc[i], 0)
                    rope_half(kT[i], pre_acc[i], 1)
                for i in range(KVL):
                    vtmp = ropool.tile([128, S], dtr, tag=f"vt{i}",
                                       name="vtmp", bufs=1)
                    nc.vector.tensor_copy(vtmp[:, 0:512],
                                          pre_acc[2 + i][:, 0:512])
                    nc.vector.tensor_copy(vtmp[:, 512:S],
                                          pre_acc[2 + i][:, 512:S])
                    vtmps.append(vtmp)

            # ---------------- head loop ----------------
            # slot schedule inside chunk `it`'s projection, group g (0..7):
            #   g=0:  fin(it-3, 6), pv(it-3, 7), qk(it-1, 0)
            #   g=1:  fin(it-3, 7), pv(it-2, 0), qk(it-1, 1)
            #   g>=2: fin(it-2, g-2), pv(it-2, g-1), qk(it-1, g)
            # so every transpose (fin) trails its PV block by two groups and
            # exp for head it-1 is paced across the whole chunk.
            from contextlib import ExitStack
            with tc.tile_pool(name="psSmall", bufs=4, space="PSUM") as psSm, \
                 tc.tile_pool(name="pt", bufs=18) as ptpool, \
                 tc.tile_pool(name="ynorm", bufs=3) as ypool, \
                 tc.tile_pool(name="recs", bufs=3) as recpool, \
                 tc.tile_pool(name="wo", bufs=3) as wopool, \
                 tc.tile_pool(name="osb", bufs=3) as opool:
                qk_stack = ExitStack()
                psProj = qk_stack.enter_context(
                    tc.tile_pool(name="psProj", bufs=2, space="PSUM"))
                pts = [[None] * TT for _ in range(HL)]
                ysbs = {}

                def qk_pair(h, kc):
                    kv = h // (HL // KVL)
                    pts[h][kc] = ptpool.tile([128, S], dtr, tag="pt",
                                             name=f"pt{h}_{kc}")
                    for tb in range(2):
                        sp = psSm.tile([128, 512], dt, tag="small", name="sp")
                        nc.tensor.matmul(
                            sp[:], kT[kv][:, kc * 128:(kc + 1) * 128],
                            qT[h][:, tb * 512:(tb + 1) * 512],
                            start=True, stop=True, skip_group_check=True)
                        nc.scalar.activation(
                            pts[h][kc][:, tb * 512:(tb + 1) * 512], sp[:],
                            mybir.ActivationFunctionType.Exp,
                            scale=float(SCALE))

                def pv_mm(h, qt):
                    kv = h // (HL // KVL)
                    yp = psSm.tile([128, 512], dt, tag="small", name="yp")
                    for kc in range(TT):
                        nc.tensor.matmul(
                            yp[:, 0:HD + 1],
                            pts[h][kc][:, qt * 128:(qt + 1) * 128],
                            v_nat[kv][kc][:],
                            start=(kc == 0), stop=(kc == TT - 1),
                            skip_group_check=True)
                    rec = recpool.tile([128, 1], dt, tag="rec", name="rec")
                    nc.vector.reciprocal(rec[:], yp[:, HD:HD + 1])
                    ysb = ypool.tile([128, HD], dtr, tag="ysb", name="ysb")
                    nc.vector.tensor_scalar_mul(ysb[:], yp[:, 0:HD], rec[:])
                    ysbs[(h, qt)] = ysb

                def pv_fin(h, qt):
                    ysb = ysbs.pop((h, qt))
                    ytp = psSm.tile([128, 128], dtr, tag="small", name="ytp")
                    nc.tensor.transpose(ytp[:], ysb[:], ident[:])
                    nc.vector.tensor_copy(yT[h][:, qt * 128:(qt + 1) * 128],
                                          ytp[:])

                extra_q = []

                def head_step(it, g, extra=None):
                    if g == 0:
                        fin_h, fin_qt = it - 3, 6
                        pv_h, pv_qt = it - 3, 7
                    elif g == 1:
                        fin_h, fin_qt = it - 3, 7
                        pv_h, pv_qt = it - 2, 0
                    else:
                        fin_h, fin_qt = it - 2, g - 2
                        pv_h, pv_qt = it - 2, g - 1
                    # qk/pv matmuls run before each fin transpose so the DVE
                    # normalize chain it depends on is always covered
                    if g > 0 and 0 <= it - 1 < HL and g < TT:
                        qk_pair(it - 1, g)
                    if extra is not None:
                        extra()
                    for _ in range(2):
                        if extra_q:
                            extra_q.pop(0)()
                    if 0 <= pv_h < HL and pv_qt < TT:
                        pv_mm(pv_h, pv_qt)
                    if 0 <= fin_h < HL and (fin_h, fin_qt) in ysbs:
                        pv_fin(fin_h, fin_qt)
                    if g == 0 and 0 <= it - 1 < HL:
                        qk_pair(it - 1, 0)

                def head_chunk(it, wt):
                    # tb-outer so the first half's rope overlaps the second
                    # half's matmuls, shortening the qT critical chain
                    acc = psProj.tile([128, S], dt, tag="acc", bufs=2,
                                      name="acc")
                    q = qpool.tile([128, S], dtr, tag="qT", name=f"qT{it}")
                    n = 0
                    for tb in range(2):
                        for ec in range(ECH):
                            nc.tensor.matmul(
                                acc[:, tb * 512:(tb + 1) * 512], wt[:, ec, :],
                                xs[:, ec, tb * 512:(tb + 1) * 512],
                                start=(ec == 0), stop=(ec == ECH - 1),
                                skip_group_check=True)
                            n += 1
                            if it >= 1 and n % 8 == 0:
                                head_step(it, n // 8 - 1)
                        if tb == 0 and it + 1 < HL:
                            wts[it + 1] = w_dma(it + 1, f"wt_q{it + 1}")
                        rope_half(q, acc, tb)
                    return q

                def v_unit(i, kt):
                    pt = psSm.tile([128, 128], dtr, tag="small", name="vtp")
                    nc.tensor.transpose(
                        pt[:], vtmps[i][:, kt * 128:(kt + 1) * 128], ident[:])
                    nc.vector.tensor_copy(v_nat[i][kt][:, 0:HD], pt[:])
                    nc.vector.memset(v_nat[i][kt][:, HD:HD + 1], 1.0)

                # V transposes slot into chunk Q1's interleave groups
                extra_q.extend(
                    lambda i=i, kt=kt: v_unit(i, kt)
                    for i in range(KVL) for kt in range(TT))
                for it in range(1, HL):
                    qT[it] = head_chunk(it, wts[it])
                # virtual iteration 8 drains QK of head 7 + PV of heads 5/6
                for g in range(TT):
                    head_step(HL, g)
                qk_stack.close()  # free psProj banks for psO

                def wo_dma(oc):
                    wt = wopool.tile([128, HL, 128], dtr, tag="wo",
                                     name=f"wt_o{oc}")
                    nc.sync.dma_start(
                        out=wt[:],
                        in_=wo_d[oc * 128:(oc + 1) * 128, :].rearrange(
                            "p (c m) -> p c m", m=128))
                    return wt

                def e_half(op, wt, oc, tb, yc_list, start, stop, ot=None):
                    for yc in yc_list:
                        nc.tensor.matmul(
                            op[:, tb * 512:(tb + 1) * 512], wt[:, yc, :],
                            yT[yc][:, tb * 512:(tb + 1) * 512],
                            start=(start and yc == yc_list[0]),
                            stop=(stop and yc == yc_list[-1]),
                            skip_group_check=True)
                    if ot is not None:
                        nc.scalar.copy(ot[:, tb * 512:(tb + 1) * 512],
                                       op[:, tb * 512:(tb + 1) * 512])
                        nc.sync.dma_start(
                            out=out_d[oc * 128:(oc + 1) * 128,
                                      tb * 512:(tb + 1) * 512],
                            in_=ot[:, tb * 512:(tb + 1) * 512])

                # ------------ out projection (partial, transposed, fp16) ----
                # oc 0/1 accumulate heads 0-5 interleaved into the PV drain of
                # heads 6/7, so the tail never idles the PE
                with tc.tile_pool(name="psO", bufs=2, space="PSUM") as psO:
                    wt_o01 = [wo_dma(0), wo_dma(1)]
                    op01 = [psO.tile([128, S], dt, tag="op", name=f"op{j}")
                            for j in range(2)]
                    ethunks = []
                    for j in range(2):
                        for tb in range(2):
                            for y0 in (0, 2, 4):
                                ethunks.append(
                                    lambda j=j, tb=tb, y0=y0: e_half(
                                        op01[j], wt_o01[j], j, tb,
                                        [y0, y0 + 1], start=(y0 == 0),
                                        stop=False))
                    # yc=6 contributions become legal once head 6 finishes
                    # (virtual iteration 9, group 1) — keep them last
                    for j in range(2):
                        for tb in range(2):
                            ethunks.append(
                                lambda j=j, tb=tb: e_half(
                                    op01[j], wt_o01[j], j, tb, [6],
                                    start=False, stop=False))

                    def extra2():
                        for _ in range(2):
                            if ethunks:
                                ethunks.pop(0)()

                    for g in range(TT):
                        head_step(HL + 1, g, extra=extra2)
                    head_step(HL + 2, 0, extra=extra2)
                    head_step(HL + 2, 1, extra=extra2)
                    while ethunks:
                        ethunks.pop(0)()
                    for j in range(2):
                        ot = opool.tile([128, S], dtr, tag="ot", name="ot")
                        for tb in range(2):
                            e_half(op01[j], wt_o01[j], j, tb, [7],
                                   start=False, stop=True, ot=ot)
                    for oc in range(2, E // 128):
                        wt = wo_dma(oc)
                        op = psO.tile([128, S], dt, tag="op", name="op")
                        ot = opool.tile([128, S], dtr, tag="ot", name="ot")
                        for tb in range(2):
                            e_half(op, wt, oc, tb, list(range(HL)),
                                   start=True, stop=True, ot=ot)

    nc.compile()
    return nc


def _rope_tables():
    inv = 1.0 / (10000.0 ** (np.arange(0, HD, 2, dtype=np.float32) / HD))  # [64]
    ang = np.arange(S, dtype=np.float32)[None, :] * inv[:, None]           # [64, S]
    cos = np.concatenate([np.cos(ang), np.cos(ang)], axis=0).astype(np.float32)   # [128, S]
    sin = np.sin(ang)
    sinp = np.concatenate([-sin, sin], axis=0).astype(np.float32)          # [128, S]
    return cos, sinp


def _rearrange_w(w, n_chunks):
    # [E_rows, n_chunks*128] -> [n_chunks*128, E_rows] blocks: row cc*128+p
    # holds w[c*128+p, cc*128+m] at col c*128+m
    e_rows = w.shape[0]
    c = e_rows // 128
    return np.ascontiguousarray(
        w.reshape(c, 128, n_chunks, 128).transpose(2, 1, 0, 3).reshape(
            n_chunks * 128, e_rows))


def make_in_maps(x, wq, wk, wv, wo):
    cos, sinp = _rope_tables()
    ndt = np.float16 if MM_DT == "float16" else np.float32
    x = np.ascontiguousarray(x, dtype=np.float32)
    in_maps = []
    for c in range(N_CORES):
        b, r = c // TP, c % TP
        in_maps.append({
            "xt": np.ascontiguousarray(x[b].T).astype(ndt),
            "wq": _rearrange_w(
                wq[:, r * QCOLS:(r + 1) * QCOLS].astype(ndt), HL),
            "wk": _rearrange_w(
                wk[:, r * KVCOLS:(r + 1) * KVCOLS].astype(ndt), KVL),
            "wv": _rearrange_w(
                wv[:, r * KVCOLS:(r + 1) * KVCOLS].astype(ndt), KVL),
            "wo": _rearrange_w(
                wo[r * QCOLS:(r + 1) * QCOLS, :].astype(ndt), ECH),
            "cos": cos.astype(ndt),
            "sinp": sinp.astype(ndt),
        })
    return in_maps


def kernel(x, wq, wk, wv, wo):
    global _PROGRAM
    from concourse.bass_utils import run_bass_kernel_spmd

    if _PROGRAM is None:
        _PROGRAM = _build_program()
    nc = _PROGRAM

    res = run_bass_kernel_spmd(nc, make_in_maps(x, wq, wk, wv, wo),
                               list(range(N_CORES)))

    out = np.zeros((B, S, E), dtype=np.float32)
    for c in range(N_CORES):
        b = c // TP
        out[b] += res.results[c]["out_t"].T.astype(np.float32)
    return out


# revision 36
# speedup vs baseline: 1.0253x; 1.0213x over previous
"""GQA (B=2,S=1024,E=4096,H=32,KV=8,HD=128, RoPE, no causal mask) on 8 NeuronCores.

Sharding: 2 batch-groups x 4-way head tensor-parallel.
Core c: batch b=c//4, tp rank r=c%4 -> 8 q heads [8r,8r+8), 2 kv heads [2r,2r+2),
wo rows [1024r, 1024(r+1)).  Each core computes a partial output
out_part = y_local @ wo[local_rows, :]  (emitted transposed as [4096, 1024] fp16);
host sums the 4 partials per batch. No device collectives needed.

v4: single fused pipeline.
- Projections are chunk-major (full-E accumulation in PSUM), order
  K0,K1,V0 interleaved per e-chunk (tracks the x DMA stream), V1, Q0..Q7.
- Head h's QK+exp / PV / y-transpose are slot-scheduled into chunk h+1 / h+2's
  projection groups so scalar-engine exp (~110us) and all DVE chains hide
  under Tensor work.
- Weights are host-prearranged so every weight DMA is contiguous per
  partition; output DMA is fp16 (host accumulates partials in fp32).
"""
import sys

sys.path.insert(0, "/opt/trn_rl_repo")

import numpy as np

B = 2
S = 1024
E = 4096
HD = 128
N_CORES = 8
TP = 4            # tensor-parallel ranks per batch group
HL = 8            # q heads per core
KVL = 2           # kv heads per core
QCOLS = HL * HD   # 1024
KVCOLS = KVL * HD  # 256
ECH = E // 128    # 32 e-chunks
TT = S // 128     # 8 token tiles
SCALE = 1.0 / np.sqrt(np.float32(HD))
MM_DT = "float16"

_PROGRAM = None


def _build_program():
    import concourse.bass as bass  # noqa: F401
    from concourse import bacc
    import concourse.mybir as mybir
    from concourse.tile import TileContext
    from concourse.masks import make_identity

    dt = mybir.dt.float32
    dtr = getattr(mybir.dt, MM_DT)
    nc = bacc.Bacc("TRN2", target_bir_lowering=False, debug=False,
                   num_devices=N_CORES)

    xt_d = nc.declare_dram_parameter("xt", [E, S], dtr, isOutput=False)
    # host-prearranged: row block cc*128+p holds w[:, cc*128:...] row c*128+p
    wq_d = nc.declare_dram_parameter("wq", [HL * 128, E], dtr, isOutput=False)
    wk_d = nc.declare_dram_parameter("wk", [KVL * 128, E], dtr, isOutput=False)
    wv_d = nc.declare_dram_parameter("wv", [KVL * 128, E], dtr, isOutput=False)
    wo_d = nc.declare_dram_parameter("wo", [ECH * 128, QCOLS], dtr,
                                     isOutput=False)
    cos_d = nc.declare_dram_parameter("cos", [HD, S], dtr, isOutput=False)
    sinp_d = nc.declare_dram_parameter("sinp", [HD, S], dtr, isOutput=False)
    out_d = nc.declare_dram_parameter("out_t", [E, S], dtr, isOutput=True)

    def w_src(cc):
        # [128, ECH, 128] view of chunk cc's weights, contiguous per partition
        if cc < HL:
            base = wq_d
        elif cc < HL + KVL:
            base, cc = wk_d, cc - HL
        else:
            base, cc = wv_d, cc - HL - KVL
        return base[cc * 128:(cc + 1) * 128, :].rearrange(
            "p (c m) -> p c m", m=128)

    with TileContext(nc) as tc:
        with tc.tile_pool(name="const", bufs=1) as cpool, \
             tc.tile_pool(name="persist", bufs=1) as ppool, \
             tc.tile_pool(name="vnat", bufs=1) as vpool, \
             tc.tile_pool(name="wstream", bufs=5) as wpool, \
             tc.tile_pool(name="qroll", bufs=3) as qpool, \
             tc.tile_pool(name="rope", bufs=2) as ropool:
            ident_f = cpool.tile([128, 128], dt)
            make_identity(nc, ident_f[:])
            ident = cpool.tile([128, 128], dtr)
            nc.scalar.copy(ident[:], ident_f[:])
            cos_t = cpool.tile([HD, S], dtr, tag="cos")
            sinp_t = cpool.tile([HD, S], dtr, tag="sinp")

            # persistent data
            xs = ppool.tile([128, ECH, S], dtr, tag="xs", name="xs")
            kT = [ppool.tile([128, S], dtr, tag=f"kT{i}", name=f"kT{i}")
                  for i in range(KVL)]
            yT = [ppool.tile([128, S], dtr, tag=f"yT{i}", name=f"yT{i}")
                  for i in range(HL)]
            v_nat = [[vpool.tile([128, HD + 1], dtr, tag=f"v{kv}_{kt}",
                                 name=f"v{kv}_{kt}")
                      for kt in range(TT)] for kv in range(KVL)]

            # DMA emission order matters: the Sync engine issues descriptors
            # in order at ~240-330GB/s aggregate, so stage the first four
            # chunks' weights per-superchunk between x slices.
            pre_cc = [HL, HL + 1, HL + KVL, HL + KVL + 1]  # K0, K1, V0, V1
            pre_wt = [wpool.tile([128, ECH, 128], dtr, tag="w",
                                 name=f"wt_pre{j}") for j in range(4)]
            for es in range(4):
                wjs = [0, 1, 2, 3] if es else [0]
                if es == 0:  # first matmul needs wt_k0[es0] + xs[0] first
                    nc.sync.dma_start(out=pre_wt[0][:, 0:8, :],
                                      in_=w_src(pre_cc[0])[:, 0:8, :])
                    nc.sync.dma_start(out=xs[:, 0, :], in_=xt_d[0:128, :])
                    wjs = [1, 2, 3]
                for j in wjs:
                    nc.sync.dma_start(
                        out=pre_wt[j][:, es * 8:(es + 1) * 8, :],
                        in_=w_src(pre_cc[j])[:, es * 8:(es + 1) * 8, :])
                for ec in range(es * 8 + (1 if es == 0 else 0), (es + 1) * 8):
                    nc.sync.dma_start(out=xs[:, ec, :],
                                      in_=xt_d[ec * 128:(ec + 1) * 128, :])
                if es == 1:
                    nc.sync.dma_start(out=cos_t[:], in_=cos_d[:])
                    nc.sync.dma_start(out=sinp_t[:], in_=sinp_d[:])

            def w_dma(cc, name):
                wt = wpool.tile([128, ECH, 128], dtr, tag="w", name=name)
                nc.sync.dma_start(out=wt[:], in_=w_src(cc)[:])
                return wt

            def rope_half(dstT, acc, tb):
                lo, hi = tb * 512, (tb + 1) * 512
                tmp = ropool.tile([HD, 512], dtr, tag=f"t0{tb}", name="tmp")
                nc.scalar.copy(tmp[:], acc[:, lo:hi])
                sh = ropool.tile([HD, 512], dtr, tag=f"sh{tb}", name="sh")
                nc.sync.dma_start(out=sh[0:64, :], in_=tmp[64:128, :])
                nc.sync.dma_start(out=sh[64:128, :], in_=tmp[0:64, :])
                t1 = ropool.tile([HD, 512], dtr, tag=f"t1{tb}", name="t1")
                nc.vector.tensor_mul(t1[:], tmp[:], cos_t[:, lo:hi])
                nc.vector.tensor_mul(sh[:], sh[:], sinp_t[:, lo:hi])
                nc.vector.tensor_add(dstT[:, lo:hi], t1[:], sh[:])

            # ---------------- pre-head phase ----------------
            # all four K/V chunks interleaved per e-chunk so compute tracks
            # the x DMA stream (4 accumulators = all 8 PSUM banks)
            vtmps = []
            qT = [None] * HL
            wts = [None] * (HL + 1)
            with tc.tile_pool(name="psPre", bufs=1, space="PSUM") as psPre:
                pre_acc = [psPre.tile([128, S], dt, tag="acc", bufs=4,
                                      name=f"accp{j}") for j in range(4)]
                for ec in range(ECH):
                    for j in range(4):
                        for tb in range(2):
                            nc.tensor.matmul(
                                pre_acc[j][:, tb * 512:(tb + 1) * 512],
                                pre_wt[j][:, ec, :],
                                xs[:, ec, tb * 512:(tb + 1) * 512],
                                start=(ec == 0), stop=(ec == ECH - 1),
                                skip_group_check=True)
                wts[0] = w_dma(0, "wt_q0")
                # V copies (DVE) first so the V transposes can fill the PE
                # while the K ropes drain this pool
                for i in range(KVL):
                    vtmp = ropool.tile([128, S], dtr, tag=f"vt{i}",
                                       name="vtmp", bufs=1)
                    nc.vector.tensor_copy(vtmp[:, 0:512],
                                          pre_acc[2 + i][:, 0:512])
                    nc.vector.tensor_copy(vtmp[:, 512:S],
                                          pre_acc[2 + i][:, 512:S])
                    vtmps.append(vtmp)
                for i in range(KVL):
                    rope_half(kT[i], pre_acc[i], 0)
                    rope_half(kT[i], pre_acc[i], 1)

            # ---------------- head loop ----------------
            # slot schedule inside chunk `it`'s projection, group g (0..7):
            #   g=0:  fin(it-3, 6), pv(it-3, 7), qk(it-1, 0)
            #   g=1:  fin(it-3, 7), pv(it-2, 0), qk(it-1, 1)
            #   g>=2: fin(it-2, g-2), pv(it-2, g-1), qk(it-1, g)
            # so every transpose (fin) trails its PV block by two groups and
            # exp for head it-1 is paced across the whole chunk.
            from contextlib import ExitStack
            with tc.tile_pool(name="psSmall", bufs=4, space="PSUM") as psSm, \
                 tc.tile_pool(name="pt", bufs=18) as ptpool, \
                 tc.tile_pool(name="ynorm", bufs=3) as ypool, \
                 tc.tile_pool(name="recs", bufs=3) as recpool, \
                 tc.tile_pool(name="wo", bufs=3) as wopool, \
                 tc.tile_pool(name="osb", bufs=3) as opool:
                qk_stack = ExitStack()
                psProj = qk_stack.enter_context(
                    tc.tile_pool(name="psProj", bufs=2, space="PSUM"))
                pts = [[None] * TT for _ in range(HL)]
                ysbs = {}

                def qk_pair(h, kc):
                    kv = h // (HL // KVL)
                    pts[h][kc] = ptpool.tile([128, S], dtr, tag="pt",
                                             name=f"pt{h}_{kc}")
                    for tb in range(2):
                        sp = psSm.tile([128, 512], dt, tag="small", name="sp")
                        nc.tensor.matmul(
                            sp[:], kT[kv][:, kc * 128:(kc + 1) * 128],
                            qT[h][:, tb * 512:(tb + 1) * 512],
                            start=True, stop=True, skip_group_check=True)
                        nc.scalar.activation(
                            pts[h][kc][:, tb * 512:(tb + 1) * 512], sp[:],
                            mybir.ActivationFunctionType.Exp,
                            scale=float(SCALE))

                def pv_mm(h, qt):
                    kv = h // (HL // KVL)
                    yp = psSm.tile([128, 512], dt, tag="small", name="yp")
                    for kc in range(TT):
                        nc.tensor.matmul(
                            yp[:, 0:HD + 1],
                            pts[h][kc][:, qt * 128:(qt + 1) * 128],
                            v_nat[kv][kc][:],
                            start=(kc == 0), stop=(kc == TT - 1),
                            skip_group_check=True)
                    rec = recpool.tile([128, 1], dt, tag="rec", name="rec")
                    nc.vector.reciprocal(rec[:], yp[:, HD:HD + 1])
                    ysb = ypool.tile([128, HD], dtr, tag="ysb", name="ysb")
                    nc.vector.tensor_scalar_mul(ysb[:], yp[:, 0:HD], rec[:])
                    ysbs[(h, qt)] = ysb

                def pv_fin(h, qt):
                    ysb = ysbs.pop((h, qt))
                    ytp = psSm.tile([128, 128], dtr, tag="small", name="ytp")
                    nc.tensor.transpose(ytp[:], ysb[:], ident[:])
                    nc.vector.tensor_copy(yT[h][:, qt * 128:(qt + 1) * 128],
                                          ytp[:])

                extra_q = []

                def head_step(it, g, extra=None):
                    if g == 0:
                        fin_h, fin_qt = it - 3, 6
                        pv_h, pv_qt = it - 3, 7
                    elif g == 1:
                        fin_h, fin_qt = it - 3, 7
                        pv_h, pv_qt = it - 2, 0
                    else:
                        fin_h, fin_qt = it - 2, g - 2
                        pv_h, pv_qt = it - 2, g - 1
                    # qk/pv matmuls run before each fin transpose so the DVE
                    # normalize chain it depends on is always covered
                    if g > 0 and 0 <= it - 1 < HL and g < TT:
                        qk_pair(it - 1, g)
                    if extra is not None:
                        extra()
                    for _ in range(2):
                        if extra_q:
                            extra_q.pop(0)()
                    if 0 <= pv_h < HL and pv_qt < TT:
                        pv_mm(pv_h, pv_qt)
                    if 0 <= fin_h < HL and (fin_h, fin_qt) in ysbs:
                        pv_fin(fin_h, fin_qt)
                    if g == 0 and 0 <= it - 1 < HL:
                        qk_pair(it - 1, 0)

                def head_chunk(it, wt):
                    # tb-outer so the first half's rope overlaps the second
                    # half's matmuls, shortening the qT critical chain
                    acc = psProj.tile([128, S], dt, tag="acc", bufs=2,
                                      name="acc")
                    q = qpool.tile([128, S], dtr, tag="qT", name=f"qT{it}")
                    n = 0
                    for tb in range(2):
                        for ec in range(ECH):
                            nc.tensor.matmul(
                                acc[:, tb * 512:(tb + 1) * 512], wt[:, ec, :],
                                xs[:, ec, tb * 512:(tb + 1) * 512],
                                start=(ec == 0), stop=(ec == ECH - 1),
                                skip_group_check=True)
                            n += 1
                            if it >= 1 and n % 8 == 0:
                                head_step(it, n // 8 - 1)
                        if tb == 0 and it + 1 < HL:
                            wts[it + 1] = w_dma(it + 1, f"wt_q{it + 1}")
                        rope_half(q, acc, tb)
                    return q

                def v_unit(i, kt):
                    pt = psSm.tile([128, 128], dtr, tag="small", name="vtp")
                    nc.tensor.transpose(
                        pt[:], vtmps[i][:, kt * 128:(kt + 1) * 128], ident[:])
                    nc.vector.tensor_copy(v_nat[i][kt][:, 0:HD], pt[:])
                    nc.vector.memset(v_nat[i][kt][:, HD:HD + 1], 1.0)

                # V transposes fill the PE while the K ropes/V casts
                # drain the pre-head pool at the pool boundary
                for i in range(KVL):
                    for kt in range(TT):
                        v_unit(i, kt)
                for it in range(HL):
                    qT[it] = head_chunk(it, wts[it])
                # virtual iteration 8 drains QK of head 7 + PV of heads 5/6
                for g in range(TT):
                    head_step(HL, g)
                qk_stack.close()  # free psProj banks for psO

                def wo_dma(oc):
                    wt = wopool.tile([128, HL, 128], dtr, tag="wo",
                                     name=f"wt_o{oc}")
                    nc.sync.dma_start(
                        out=wt[:],
                        in_=wo_d[oc * 128:(oc + 1) * 128, :].rearrange(
                            "p (c m) -> p c m", m=128))
                    return wt

                def e_half(op, wt, oc, tb, yc_list, start, stop, ot=None):
                    for yc in yc_list:
                        nc.tensor.matmul(
                            op[:, tb * 512:(tb + 1) * 512], wt[:, yc, :],
                            yT[yc][:, tb * 512:(tb + 1) * 512],
                            start=(start and yc == yc_list[0]),
                            stop=(stop and yc == yc_list[-1]),
                            skip_group_check=True)
                    if ot is not None:
                        nc.scalar.copy(ot[:, tb * 512:(tb + 1) * 512],
                                       op[:, tb * 512:(tb + 1) * 512])
                        nc.sync.dma_start(
                            out=out_d[oc * 128:(oc + 1) * 128,
                                      tb * 512:(tb + 1) * 512],
                            in_=ot[:, tb * 512:(tb + 1) * 512])

                # ------------ out projection (partial, transposed, fp16) ----
                # oc 0/1 accumulate heads 0-5 interleaved into the PV drain of
                # heads 6/7, so the tail never idles the PE
                with tc.tile_pool(name="psO", bufs=2, space="PSUM") as psO:
                    wt_o01 = [wo_dma(0), wo_dma(1)]
                    op01 = [psO.tile([128, S], dt, tag="op", name=f"op{j}")
                            for j in range(2)]
                    ethunks = []
                    for j in range(2):
                        for tb in range(2):
                            for y0 in (0, 2, 4):
                                ethunks.append(
                                    lambda j=j, tb=tb, y0=y0: e_half(
                                        op01[j], wt_o01[j], j, tb,
                                        [y0, y0 + 1], start=(y0 == 0),
                                        stop=False))
                    # yc=6 contributions become legal once head 6 finishes
                    # (virtual iteration 9, group 1) — keep them last
                    for j in range(2):
                        for tb in range(2):
                            ethunks.append(
                                lambda j=j, tb=tb: e_half(
                                    op01[j], wt_o01[j], j, tb, [6],
                                    start=False, stop=False))

                    def extra2():
                        for _ in range(2):
                            if ethunks:
                                ethunks.pop(0)()

                    for g in range(TT):
                        head_step(HL + 1, g, extra=extra2)
                    head_step(HL + 2, 0, extra=extra2)
                    head_step(HL + 2, 1, extra=extra2)
                    while ethunks:
                        ethunks.pop(0)()
                    for j in range(2):
                        ot = opool.tile([128, S], dtr, tag="ot", name="ot")
                        for tb in range(2):
                            e_half(op01[j], wt_o01[j], j, tb, [7],
                                   start=False, stop=True, ot=ot)
                    for oc in range(2, E // 128):
                        wt = wo_dma(oc)
                        op = psO.tile([128, S], dt, tag="op", name="op")
                        ot = opool.tile([128, S], dtr, tag="ot", name="ot")
                        for tb in range(2):
                            e_half(op, wt, oc, tb, list(range(HL)),
                                   start=True, stop=True, ot=ot)

    nc.compile()
    return nc


def _rope_tables():
    inv = 1.0 / (10000.0 ** (np.arange(0, HD, 2, dtype=np.float32) / HD))  # [64]
    ang = np.arange(S, dtype=np.float32)[None, :] * inv[:, None]           # [64, S]
    cos = np.concatenate([np.cos(ang), np.cos(ang)], axis=0).astype(np.float32)   # [128, S]
    sin = np.sin(ang)
    sinp = np.concatenate([-sin, sin], axis=0).astype(np.float32)          # [128, S]
    return cos, sinp


def _rearrange_w(w, n_chunks):
    # [E_rows, n_chunks*128] -> [n_chunks*128, E_rows] blocks: row cc*128+p
    # holds w[c*128+p, cc*128+m] at col c*128+m
    e_rows = w.shape[0]
    c = e_rows // 128
    return np.ascontiguousarray(
        w.reshape(c, 128, n_chunks, 128).transpose(2, 1, 0, 3).reshape(
            n_chunks * 128, e_rows))


def make_in_maps(x, wq, wk, wv, wo):
    cos, sinp = _rope_tables()
    ndt = np.float16 if MM_DT == "float16" else np.float32
    x = np.ascontiguousarray(x, dtype=np.float32)
    in_maps = []
    for c in range(N_CORES):
        b, r = c // TP, c % TP
        in_maps.append({
            "xt": np.ascontiguousarray(x[b].T).astype(ndt),
            "wq": _rearrange_w(
                wq[:, r * QCOLS:(r + 1) * QCOLS].astype(ndt), HL),
            "wk": _rearrange_w(
                wk[:, r * KVCOLS:(r + 1) * KVCOLS].astype(ndt), KVL),
            "wv": _rearrange_w(
                wv[:, r * KVCOLS:(r + 1) * KVCOLS].astype(ndt), KVL),
            "wo": _rearrange_w(
                wo[r * QCOLS:(r + 1) * QCOLS, :].astype(ndt), ECH),
            "cos": cos.astype(ndt),
            "sinp": sinp.astype(ndt),
        })
    return in_maps


def kernel(x, wq, wk, wv, wo):
    global _PROGRAM
    from concourse.bass_utils import run_bass_kernel_spmd

    if _PROGRAM is None:
        _PROGRAM = _build_program()
    nc = _PROGRAM

    res = run_bass_kernel_spmd(nc, make_in_maps(x, wq, wk, wv, wo),
                               list(range(N_CORES)))

    out = np.zeros((B, S, E), dtype=np.float32)
    for c in range(N_CORES):
        b = c // TP
        out[b] += res.results[c]["out_t"].T.astype(np.float32)
    return out


# revision 37
# speedup vs baseline: 1.0267x; 1.0013x over previous
"""GQA (B=2,S=1024,E=4096,H=32,KV=8,HD=128, RoPE, no causal mask) on 8 NeuronCores.

Sharding: 2 batch-groups x 4-way head tensor-parallel.
Core c: batch b=c//4, tp rank r=c%4 -> 8 q heads [8r,8r+8), 2 kv heads [2r,2r+2),
wo rows [1024r, 1024(r+1)).  Each core computes a partial output
out_part = y_local @ wo[local_rows, :]  (emitted transposed as [4096, 1024] fp16);
host sums the 4 partials per batch. No device collectives needed.

v4: single fused pipeline.
- Projections are chunk-major (full-E accumulation in PSUM), order
  K0,K1,V0 interleaved per e-chunk (tracks the x DMA stream), V1, Q0..Q7.
- Head h's QK+exp / PV / y-transpose are slot-scheduled into chunk h+1 / h+2's
  projection groups so scalar-engine exp (~110us) and all DVE chains hide
  under Tensor work.
- Weights are host-prearranged so every weight DMA is contiguous per
  partition; output DMA is fp16 (host accumulates partials in fp32).
"""
import sys

sys.path.insert(0, "/opt/trn_rl_repo")

import numpy as np

B = 2
S = 1024
E = 4096
HD = 128
N_CORES = 8
TP = 4            # tensor-parallel ranks per batch group
HL = 8            # q heads per core
KVL = 2           # kv heads per core
QCOLS = HL * HD   # 1024
KVCOLS = KVL * HD  # 256
ECH = E // 128    # 32 e-chunks
TT = S // 128     # 8 token tiles
SCALE = 1.0 / np.sqrt(np.float32(HD))
MM_DT = "float16"

_PROGRAM = None


def _build_program():
    import concourse.bass as bass  # noqa: F401
    from concourse import bacc
    import concourse.mybir as mybir
    from concourse.tile import TileContext
    from concourse.masks import make_identity

    dt = mybir.dt.float32
    dtr = getattr(mybir.dt, MM_DT)
    nc = bacc.Bacc("TRN2", target_bir_lowering=False, debug=False,
                   num_devices=N_CORES)

    xt_d = nc.declare_dram_parameter("xt", [E, S], dtr, isOutput=False)
    # host-prearranged: row block cc*128+p holds w[:, cc*128:...] row c*128+p
    wq_d = nc.declare_dram_parameter("wq", [HL * 128, E], dtr, isOutput=False)
    wk_d = nc.declare_dram_parameter("wk", [KVL * 128, E], dtr, isOutput=False)
    wv_d = nc.declare_dram_parameter("wv", [KVL * 128, E], dtr, isOutput=False)
    wo_d = nc.declare_dram_parameter("wo", [ECH * 128, QCOLS], dtr,
                                     isOutput=False)
    cos_d = nc.declare_dram_parameter("cos", [HD, S], dtr, isOutput=False)
    sinp_d = nc.declare_dram_parameter("sinp", [HD, S], dtr, isOutput=False)
    out_d = nc.declare_dram_parameter("out_t", [E, S], dtr, isOutput=True)

    def w_src(cc):
        # [128, ECH, 128] view of chunk cc's weights, contiguous per partition
        if cc < HL:
            base = wq_d
        elif cc < HL + KVL:
            base, cc = wk_d, cc - HL
        else:
            base, cc = wv_d, cc - HL - KVL
        return base[cc * 128:(cc + 1) * 128, :].rearrange(
            "p (c m) -> p c m", m=128)

    with TileContext(nc) as tc:
        with tc.tile_pool(name="const", bufs=1) as cpool, \
             tc.tile_pool(name="persist", bufs=1) as ppool, \
             tc.tile_pool(name="vnat", bufs=1) as vpool, \
             tc.tile_pool(name="wstream", bufs=5) as wpool, \
             tc.tile_pool(name="qroll", bufs=3) as qpool, \
             tc.tile_pool(name="rope", bufs=2) as ropool:
            ident_f = cpool.tile([128, 128], dt)
            make_identity(nc, ident_f[:])
            ident = cpool.tile([128, 128], dtr)
            nc.scalar.copy(ident[:], ident_f[:])
            cos_t = cpool.tile([HD, S], dtr, tag="cos")
            sinp_t = cpool.tile([HD, S], dtr, tag="sinp")

            # persistent data
            xs = ppool.tile([128, ECH, S], dtr, tag="xs", name="xs")
            kT = [ppool.tile([128, S], dtr, tag=f"kT{i}", name=f"kT{i}")
                  for i in range(KVL)]
            yT = [ppool.tile([128, S], dtr, tag=f"yT{i}", name=f"yT{i}")
                  for i in range(HL)]
            v_nat = [[vpool.tile([128, HD + 1], dtr, tag=f"v{kv}_{kt}",
                                 name=f"v{kv}_{kt}")
                      for kt in range(TT)] for kv in range(KVL)]

            # DMA emission order matters: the Sync engine issues descriptors
            # in order at ~240-330GB/s aggregate, so stage the first four
            # chunks' weights per-superchunk between x slices.
            pre_cc = [HL, HL + 1, HL + KVL, HL + KVL + 1]  # K0, K1, V0, V1
            pre_wt = [wpool.tile([128, ECH, 128], dtr, tag="w",
                                 name=f"wt_pre{j}") for j in range(4)]
            for es in range(4):
                wjs = [0, 1, 2, 3] if es else [0]
                if es == 0:  # first matmul needs wt_k0[es0] + xs[0] first
                    nc.sync.dma_start(out=pre_wt[0][:, 0:8, :],
                                      in_=w_src(pre_cc[0])[:, 0:8, :])
                    nc.sync.dma_start(out=xs[:, 0, :], in_=xt_d[0:128, :])
                    wjs = [1, 2, 3]
                for j in wjs:
                    nc.sync.dma_start(
                        out=pre_wt[j][:, es * 8:(es + 1) * 8, :],
                        in_=w_src(pre_cc[j])[:, es * 8:(es + 1) * 8, :])
                for ec in range(es * 8 + (1 if es == 0 else 0), (es + 1) * 8):
                    nc.sync.dma_start(out=xs[:, ec, :],
                                      in_=xt_d[ec * 128:(ec + 1) * 128, :])
                if es == 1:
                    nc.sync.dma_start(out=cos_t[:], in_=cos_d[:])
                    nc.sync.dma_start(out=sinp_t[:], in_=sinp_d[:])

            def w_dma(cc, name):
                wt = wpool.tile([128, ECH, 128], dtr, tag="w", name=name)
                nc.sync.dma_start(out=wt[:], in_=w_src(cc)[:])
                return wt

            def rope_half(dstT, acc, tb):
                lo, hi = tb * 512, (tb + 1) * 512
                tmp = ropool.tile([HD, 512], dtr, tag=f"t0{tb}", name="tmp")
                nc.scalar.copy(tmp[:], acc[:, lo:hi])
                sh = ropool.tile([HD, 512], dtr, tag=f"sh{tb}", name="sh")
                nc.sync.dma_start(out=sh[0:64, :], in_=tmp[64:128, :])
                nc.sync.dma_start(out=sh[64:128, :], in_=tmp[0:64, :])
                t1 = ropool.tile([HD, 512], dtr, tag=f"t1{tb}", name="t1")
                nc.vector.tensor_mul(t1[:], tmp[:], cos_t[:, lo:hi])
                nc.vector.tensor_mul(sh[:], sh[:], sinp_t[:, lo:hi])
                nc.vector.tensor_add(dstT[:, lo:hi], t1[:], sh[:])

            # ---------------- pre-head phase ----------------
            # all four K/V chunks interleaved per e-chunk so compute tracks
            # the x DMA stream (4 accumulators = all 8 PSUM banks)
            vtmps = []
            qT = [None] * HL
            wts = [None] * (HL + 1)
            with tc.tile_pool(name="psPre", bufs=1, space="PSUM") as psPre:
                pre_acc = [psPre.tile([128, S], dt, tag="acc", bufs=4,
                                      name=f"accp{j}") for j in range(4)]
                for ec in range(ECH):
                    for j in range(4):
                        for tb in range(2):
                            nc.tensor.matmul(
                                pre_acc[j][:, tb * 512:(tb + 1) * 512],
                                pre_wt[j][:, ec, :],
                                xs[:, ec, tb * 512:(tb + 1) * 512],
                                start=(ec == 0), stop=(ec == ECH - 1),
                                skip_group_check=True)
                wts[0] = w_dma(0, "wt_q0")
                # V copies (DVE) first so the V transposes can fill the PE
                # while the K ropes drain this pool
                for i in range(KVL):
                    vtmp = ropool.tile([128, S], dtr, tag=f"vt{i}",
                                       name="vtmp", bufs=1)
                    nc.vector.tensor_copy(vtmp[:, 0:512],
                                          pre_acc[2 + i][:, 0:512])
                    nc.vector.tensor_copy(vtmp[:, 512:S],
                                          pre_acc[2 + i][:, 512:S])
                    vtmps.append(vtmp)
                for i in range(KVL):
                    rope_half(kT[i], pre_acc[i], 0)
                    rope_half(kT[i], pre_acc[i], 1)

            # ---------------- head loop ----------------
            # slot schedule inside chunk `it`'s projection, group g (0..7):
            #   g=0:  fin(it-3, 6), pv(it-3, 7), qk(it-1, 0)
            #   g=1:  fin(it-3, 7), pv(it-2, 0), qk(it-1, 1)
            #   g>=2: fin(it-2, g-2), pv(it-2, g-1), qk(it-1, g)
            # so every transpose (fin) trails its PV block by two groups and
            # exp for head it-1 is paced across the whole chunk.
            from contextlib import ExitStack
            with tc.tile_pool(name="psSmall", bufs=4, space="PSUM") as psSm, \
                 tc.tile_pool(name="pt", bufs=18) as ptpool, \
                 tc.tile_pool(name="ynorm", bufs=3) as ypool, \
                 tc.tile_pool(name="recs", bufs=3) as recpool, \
                 tc.tile_pool(name="wo", bufs=3) as wopool, \
                 tc.tile_pool(name="osb", bufs=3) as opool:
                qk_stack = ExitStack()
                psProj = qk_stack.enter_context(
                    tc.tile_pool(name="psProj", bufs=2, space="PSUM"))
                pts = [[None] * TT for _ in range(HL)]
                ysbs = {}

                def qk_pair(h, kc):
                    kv = h // (HL // KVL)
                    pts[h][kc] = ptpool.tile([128, S], dtr, tag="pt",
                                             name=f"pt{h}_{kc}")
                    for tb in range(2):
                        sp = psSm.tile([128, 512], dt, tag="small", name="sp")
                        nc.tensor.matmul(
                            sp[:], kT[kv][:, kc * 128:(kc + 1) * 128],
                            qT[h][:, tb * 512:(tb + 1) * 512],
                            start=True, stop=True, skip_group_check=True)
                        nc.scalar.activation(
                            pts[h][kc][:, tb * 512:(tb + 1) * 512], sp[:],
                            mybir.ActivationFunctionType.Exp,
                            scale=float(SCALE))

                def pv_mm(h, qt):
                    kv = h // (HL // KVL)
                    yp = psSm.tile([128, 512], dt, tag="small", name="yp")
                    for kc in range(TT):
                        nc.tensor.matmul(
                            yp[:, 0:HD + 1],
                            pts[h][kc][:, qt * 128:(qt + 1) * 128],
                            v_nat[kv][kc][:],
                            start=(kc == 0), stop=(kc == TT - 1),
                            skip_group_check=True)
                    rec = recpool.tile([128, 1], dt, tag="rec", name="rec")
                    nc.vector.reciprocal(rec[:], yp[:, HD:HD + 1])
                    ysb = ypool.tile([128, HD], dtr, tag="ysb", name="ysb")
                    nc.vector.tensor_scalar_mul(ysb[:], yp[:, 0:HD], rec[:])
                    ysbs[(h, qt)] = ysb

                def pv_fin(h, qt):
                    ysb = ysbs.pop((h, qt))
                    ytp = psSm.tile([128, 128], dtr, tag="small", name="ytp")
                    nc.tensor.transpose(ytp[:], ysb[:], ident[:])
                    nc.vector.tensor_copy(yT[h][:, qt * 128:(qt + 1) * 128],
                                          ytp[:])

                extra_q = []

                def head_step(it, g, extra=None):
                    if g == 0:
                        fin_h, fin_qt = it - 3, 6
                        pv_h, pv_qt = it - 3, 7
                    elif g == 1:
                        fin_h, fin_qt = it - 3, 7
                        pv_h, pv_qt = it - 2, 0
                    else:
                        fin_h, fin_qt = it - 2, g - 2
                        pv_h, pv_qt = it - 2, g - 1
                    # qk/pv matmuls run before each fin transpose so the DVE
                    # normalize chain it depends on is always covered
                    if g > 0 and 0 <= it - 1 < HL and g < TT:
                        qk_pair(it - 1, g)
                    if extra is not None:
                        extra()
                    for _ in range(2):
                        if extra_q:
                            extra_q.pop(0)()
                    if 0 <= pv_h < HL and pv_qt < TT:
                        pv_mm(pv_h, pv_qt)
                    if 0 <= fin_h < HL and (fin_h, fin_qt) in ysbs:
                        pv_fin(fin_h, fin_qt)
                    if g == 0 and 0 <= it - 1 < HL:
                        qk_pair(it - 1, 0)

                def head_chunk(it, wt):
                    # tb-outer so the first half's rope overlaps the second
                    # half's matmuls, shortening the qT critical chain
                    acc = psProj.tile([128, S], dt, tag="acc", bufs=2,
                                      name="acc")
                    q = qpool.tile([128, S], dtr, tag="qT", name=f"qT{it}")
                    n = 0
                    for tb in range(2):
                        for ec in range(ECH):
                            nc.tensor.matmul(
                                acc[:, tb * 512:(tb + 1) * 512], wt[:, ec, :],
                                xs[:, ec, tb * 512:(tb + 1) * 512],
                                start=(ec == 0), stop=(ec == ECH - 1),
                                skip_group_check=True)
                            n += 1
                            if it >= 1 and n % 8 == 0:
                                head_step(it, n // 8 - 1)
                        if tb == 0 and it + 1 < HL:
                            wts[it + 1] = w_dma(it + 1, f"wt_q{it + 1}")
                        rope_half(q, acc, tb)
                    return q

                def v_unit(i, kt):
                    pt = psSm.tile([128, 128], dtr, tag="small", name="vtp")
                    nc.tensor.transpose(
                        pt[:], vtmps[i][:, kt * 128:(kt + 1) * 128], ident[:])
                    nc.vector.tensor_copy(v_nat[i][kt][:, 0:HD], pt[:])
                    nc.vector.memset(v_nat[i][kt][:, HD:HD + 1], 1.0)

                # V transposes: half fill the PE at the pool boundary,
                # half cover iteration 1's exposed rope wait
                for kt in range(TT):
                    v_unit(0, kt)
                extra_q.extend(lambda kt=kt: v_unit(1, kt)
                               for kt in range(TT))
                for it in range(HL):
                    qT[it] = head_chunk(it, wts[it])
                # virtual iteration 8 drains QK of head 7 + PV of heads 5/6
                for g in range(TT):
                    head_step(HL, g)
                qk_stack.close()  # free psProj banks for psO

                def wo_dma(oc):
                    wt = wopool.tile([128, HL, 128], dtr, tag="wo",
                                     name=f"wt_o{oc}")
                    nc.sync.dma_start(
                        out=wt[:],
                        in_=wo_d[oc * 128:(oc + 1) * 128, :].rearrange(
                            "p (c m) -> p c m", m=128))
                    return wt

                def e_half(op, wt, oc, tb, yc_list, start, stop, ot=None):
                    for yc in yc_list:
                        nc.tensor.matmul(
                            op[:, tb * 512:(tb + 1) * 512], wt[:, yc, :],
                            yT[yc][:, tb * 512:(tb + 1) * 512],
                            start=(start and yc == yc_list[0]),
                            stop=(stop and yc == yc_list[-1]),
                            skip_group_check=True)
                    if ot is not None:
                        nc.scalar.copy(ot[:, tb * 512:(tb + 1) * 512],
                                       op[:, tb * 512:(tb + 1) * 512])
                        nc.sync.dma_start(
                            out=out_d[oc * 128:(oc + 1) * 128,
                                      tb * 512:(tb + 1) * 512],
                            in_=ot[:, tb * 512:(tb + 1) * 512])

                # ------------ out projection (partial, transposed, fp16) ----
                # oc 0/1 accumulate heads 0-5 interleaved into the PV drain of
                # heads 6/7, so the tail never idles the PE
                with tc.tile_pool(name="psO", bufs=2, space="PSUM") as psO:
                    wt_o01 = [wo_dma(0), wo_dma(1)]
                    op01 = [psO.tile([128, S], dt, tag="op", name=f"op{j}")
                            for j in range(2)]
                    ethunks = []
                    for j in range(2):
                        for tb in range(2):
                            for y0 in (0, 2, 4):
                                ethunks.append(
                                    lambda j=j, tb=tb, y0=y0: e_half(
                                        op01[j], wt_o01[j], j, tb,
                                        [y0, y0 + 1], start=(y0 == 0),
                                        stop=False))
                    # yc=6 contributions become legal once head 6 finishes
                    # (virtual iteration 9, group 1) — keep them last
                    for j in range(2):
                        for tb in range(2):
                            ethunks.append(
                                lambda j=j, tb=tb: e_half(
                                    op01[j], wt_o01[j], j, tb, [6],
                                    start=False, stop=False))

                    def extra2():
                        for _ in range(2):
                            if ethunks:
                                ethunks.pop(0)()

                    for g in range(TT):
                        head_step(HL + 1, g, extra=extra2)
                    head_step(HL + 2, 0, extra=extra2)
                    head_step(HL + 2, 1, extra=extra2)
                    while ethunks:
                        ethunks.pop(0)()
                    for j in range(2):
                        ot = opool.tile([128, S], dtr, tag="ot", name="ot")
                        for tb in range(2):
                            e_half(op01[j], wt_o01[j], j, tb, [7],
                                   start=False, stop=True, ot=ot)
                    for oc in range(2, E // 128):
                        wt = wo_dma(oc)
                        op = psO.tile([128, S], dt, tag="op", name="op")
                        ot = opool.tile([128, S], dtr, tag="ot", name="ot")
                        for tb in range(2):
                            e_half(op, wt, oc, tb, list(range(HL)),
                                   start=True, stop=True, ot=ot)

    nc.compile()
    return nc


def _rope_tables():
    inv = 1.0 / (10000.0 ** (np.arange(0, HD, 2, dtype=np.float32) / HD))  # [64]
    ang = np.arange(S, dtype=np.float32)[None, :] * inv[:, None]           # [64, S]
    cos = np.concatenate([np.cos(ang), np.cos(ang)], axis=0).astype(np.float32)   # [128, S]
    sin = np.sin(ang)
    sinp = np.concatenate([-sin, sin], axis=0).astype(np.float32)          # [128, S]
    return cos, sinp


def _rearrange_w(w, n_chunks):
    # [E_rows, n_chunks*128] -> [n_chunks*128, E_rows] blocks: row cc*128+p
    # holds w[c*128+p, cc*128+m] at col c*128+m
    e_rows = w.shape[0]
    c = e_rows // 128
    return np.ascontiguousarray(
        w.reshape(c, 128, n_chunks, 128).transpose(2, 1, 0, 3).reshape(
            n_chunks * 128, e_rows))


def make_in_maps(x, wq, wk, wv, wo):
    cos, sinp = _rope_tables()
    ndt = np.float16 if MM_DT == "float16" else np.float32
    x = np.ascontiguousarray(x, dtype=np.float32)
    in_maps = []
    for c in range(N_CORES):
        b, r = c // TP, c % TP
        in_maps.append({
            "xt": np.ascontiguousarray(x[b].T).astype(ndt),
            "wq": _rearrange_w(
                wq[:, r * QCOLS:(r + 1) * QCOLS].astype(ndt), HL),
            "wk": _rearrange_w(
                wk[:, r * KVCOLS:(r + 1) * KVCOLS].astype(ndt), KVL),
            "wv": _rearrange_w(
                wv[:, r * KVCOLS:(r + 1) * KVCOLS].astype(ndt), KVL),
            "wo": _rearrange_w(
                wo[r * QCOLS:(r + 1) * QCOLS, :].astype(ndt), ECH),
            "cos": cos.astype(ndt),
            "sinp": sinp.astype(ndt),
        })
    return in_maps


def kernel(x, wq, wk, wv, wo):
    global _PROGRAM
    from concourse.bass_utils import run_bass_kernel_spmd

    if _PROGRAM is None:
        _PROGRAM = _build_program()
    nc = _PROGRAM

    res = run_bass_kernel_spmd(nc, make_in_maps(x, wq, wk, wv, wo),
                               list(range(N_CORES)))

    out = np.zeros((B, S, E), dtype=np.float32)
    for c in range(N_CORES):
        b = c // TP
        out[b] += res.results[c]["out_t"].T.astype(np.float32)
    return out


# revision 38
# speedup vs baseline: 1.0327x; 1.0059x over previous
"""GQA (B=2,S=1024,E=4096,H=32,KV=8,HD=128, RoPE, no causal mask) on 8 NeuronCores.

Sharding: 2 batch-groups x 4-way head tensor-parallel.
Core c: batch b=c//4, tp rank r=c%4 -> 8 q heads [8r,8r+8), 2 kv heads [2r,2r+2),
wo rows [1024r, 1024(r+1)).  Each core computes a partial output
out_part = y_local @ wo[local_rows, :]  (emitted transposed as [4096, 1024] fp16);
host sums the 4 partials per batch. No device collectives needed.

v4: single fused pipeline.
- Projections are chunk-major (full-E accumulation in PSUM), order
  K0,K1,V0 interleaved per e-chunk (tracks the x DMA stream), V1, Q0..Q7.
- Head h's QK+exp / PV / y-transpose are slot-scheduled into chunk h+1 / h+2's
  projection groups so scalar-engine exp (~110us) and all DVE chains hide
  under Tensor work.
- Weights are host-prearranged so every weight DMA is contiguous per
  partition; output DMA is fp16 (host accumulates partials in fp32).
"""
import sys

sys.path.insert(0, "/opt/trn_rl_repo")

import numpy as np

B = 2
S = 1024
E = 4096
HD = 128
N_CORES = 8
TP = 4            # tensor-parallel ranks per batch group
HL = 8            # q heads per core
KVL = 2           # kv heads per core
QCOLS = HL * HD   # 1024
KVCOLS = KVL * HD  # 256
ECH = E // 128    # 32 e-chunks
TT = S // 128     # 8 token tiles
SCALE = 1.0 / np.sqrt(np.float32(HD))
MM_DT = "float16"

_PROGRAM = None


def _build_program():
    import concourse.bass as bass  # noqa: F401
    from concourse import bacc
    import concourse.mybir as mybir
    from concourse.tile import TileContext
    from concourse.masks import make_identity

    dt = mybir.dt.float32
    dtr = getattr(mybir.dt, MM_DT)
    nc = bacc.Bacc("TRN2", target_bir_lowering=False, debug=False,
                   num_devices=N_CORES)

    xt_d = nc.declare_dram_parameter("xt", [E, S], dtr, isOutput=False)
    # host-prearranged: row block cc*128+p holds w[:, cc*128:...] row c*128+p
    wq_d = nc.declare_dram_parameter("wq", [HL * 128, E], dtr, isOutput=False)
    wk_d = nc.declare_dram_parameter("wk", [KVL * 128, E], dtr, isOutput=False)
    wv_d = nc.declare_dram_parameter("wv", [KVL * 128, E], dtr, isOutput=False)
    wo_d = nc.declare_dram_parameter("wo", [ECH * 128, QCOLS], dtr,
                                     isOutput=False)
    cos_d = nc.declare_dram_parameter("cos", [HD, S], dtr, isOutput=False)
    sinp_d = nc.declare_dram_parameter("sinp", [HD, S], dtr, isOutput=False)
    out_d = nc.declare_dram_parameter("out_t", [E, S], dtr, isOutput=True)

    def w_src(cc):
        # [128, ECH, 128] view of chunk cc's weights, contiguous per partition
        if cc < HL:
            base = wq_d
        elif cc < HL + KVL:
            base, cc = wk_d, cc - HL
        else:
            base, cc = wv_d, cc - HL - KVL
        return base[cc * 128:(cc + 1) * 128, :].rearrange(
            "p (c m) -> p c m", m=128)

    with TileContext(nc) as tc:
        with tc.tile_pool(name="const", bufs=1) as cpool, \
             tc.tile_pool(name="persist", bufs=1) as ppool, \
             tc.tile_pool(name="vnat", bufs=1) as vpool, \
             tc.tile_pool(name="wstream", bufs=5) as wpool, \
             tc.tile_pool(name="qroll", bufs=3) as qpool, \
             tc.tile_pool(name="rope", bufs=2) as ropool:
            ident_f = cpool.tile([128, 128], dt)
            make_identity(nc, ident_f[:])
            ident = cpool.tile([128, 128], dtr)
            nc.scalar.copy(ident[:], ident_f[:])
            cos_t = cpool.tile([HD, S], dtr, tag="cos")
            sinp_t = cpool.tile([HD, S], dtr, tag="sinp")

            # persistent data
            xs = ppool.tile([128, ECH, S], dtr, tag="xs", name="xs")
            kT = [ppool.tile([128, S], dtr, tag=f"kT{i}", name=f"kT{i}")
                  for i in range(KVL)]
            yT = [ppool.tile([128, S], dtr, tag=f"yT{i}", name=f"yT{i}")
                  for i in range(HL)]
            v_nat = [[vpool.tile([128, HD + 1], dtr, tag=f"v{kv}_{kt}",
                                 name=f"v{kv}_{kt}")
                      for kt in range(TT)] for kv in range(KVL)]

            # DMA emission order matters: the Sync engine issues descriptors
            # in order at ~240-330GB/s aggregate, so stage the first four
            # chunks' weights per-superchunk between x slices.
            pre_cc = [HL, HL + 1, HL + KVL, HL + KVL + 1]  # K0, K1, V0, V1
            pre_wt = [wpool.tile([128, ECH, 128], dtr, tag="w",
                                 name=f"wt_pre{j}") for j in range(4)]
            for es in range(4):
                if es == 0:  # alternate weight quarters and x slices so the
                    # first few matmuls of every chunk have data earliest
                    for j in range(4):
                        nc.sync.dma_start(out=pre_wt[j][:, 0:8, :],
                                          in_=w_src(pre_cc[j])[:, 0:8, :])
                        nc.sync.dma_start(out=xs[:, j, :],
                                          in_=xt_d[j * 128:(j + 1) * 128, :])
                    ecs = range(4, 8)
                else:
                    for j in range(4):
                        nc.sync.dma_start(
                            out=pre_wt[j][:, es * 8:(es + 1) * 8, :],
                            in_=w_src(pre_cc[j])[:, es * 8:(es + 1) * 8, :])
                    ecs = range(es * 8, (es + 1) * 8)
                for ec in ecs:
                    nc.sync.dma_start(out=xs[:, ec, :],
                                      in_=xt_d[ec * 128:(ec + 1) * 128, :])
                if es == 1:
                    nc.sync.dma_start(out=cos_t[:], in_=cos_d[:])
                    nc.sync.dma_start(out=sinp_t[:], in_=sinp_d[:])

            def w_dma(cc, name):
                wt = wpool.tile([128, ECH, 128], dtr, tag="w", name=name)
                nc.sync.dma_start(out=wt[:], in_=w_src(cc)[:])
                return wt

            def rope_half(dstT, acc, tb):
                lo, hi = tb * 512, (tb + 1) * 512
                tmp = ropool.tile([HD, 512], dtr, tag=f"t0{tb}", name="tmp")
                nc.scalar.copy(tmp[:], acc[:, lo:hi])
                sh = ropool.tile([HD, 512], dtr, tag=f"sh{tb}", name="sh")
                nc.sync.dma_start(out=sh[0:64, :], in_=tmp[64:128, :])
                nc.sync.dma_start(out=sh[64:128, :], in_=tmp[0:64, :])
                t1 = ropool.tile([HD, 512], dtr, tag=f"t1{tb}", name="t1")
                nc.vector.tensor_mul(t1[:], tmp[:], cos_t[:, lo:hi])
                nc.vector.tensor_mul(sh[:], sh[:], sinp_t[:, lo:hi])
                nc.vector.tensor_add(dstT[:, lo:hi], t1[:], sh[:])

            # ---------------- pre-head phase ----------------
            # all four K/V chunks interleaved per e-chunk so compute tracks
            # the x DMA stream (4 accumulators = all 8 PSUM banks)
            vtmps = []
            qT = [None] * HL
            wts = [None] * (HL + 1)
            with tc.tile_pool(name="psPre", bufs=1, space="PSUM") as psPre:
                pre_acc = [psPre.tile([128, S], dt, tag="acc", bufs=4,
                                      name=f"accp{j}") for j in range(4)]
                for ec in range(ECH):
                    for j in range(4):
                        for tb in range(2):
                            nc.tensor.matmul(
                                pre_acc[j][:, tb * 512:(tb + 1) * 512],
                                pre_wt[j][:, ec, :],
                                xs[:, ec, tb * 512:(tb + 1) * 512],
                                start=(ec == 0), stop=(ec == ECH - 1),
                                skip_group_check=True)
                wts[0] = w_dma(0, "wt_q0")
                # V copies (DVE) first so the V transposes can fill the PE
                # while the K ropes drain this pool
                for i in range(KVL):
                    vtmp = ropool.tile([128, S], dtr, tag=f"vt{i}",
                                       name="vtmp", bufs=1)
                    nc.vector.tensor_copy(vtmp[:, 0:512],
                                          pre_acc[2 + i][:, 0:512])
                    nc.vector.tensor_copy(vtmp[:, 512:S],
                                          pre_acc[2 + i][:, 512:S])
                    vtmps.append(vtmp)
                for i in range(KVL):
                    rope_half(kT[i], pre_acc[i], 0)
                    rope_half(kT[i], pre_acc[i], 1)

            # ---------------- head loop ----------------
            # slot schedule inside chunk `it`'s projection, group g (0..7):
            #   g=0:  fin(it-3, 6), pv(it-3, 7), qk(it-1, 0)
            #   g=1:  fin(it-3, 7), pv(it-2, 0), qk(it-1, 1)
            #   g>=2: fin(it-2, g-2), pv(it-2, g-1), qk(it-1, g)
            # so every transpose (fin) trails its PV block by two groups and
            # exp for head it-1 is paced across the whole chunk.
            from contextlib import ExitStack
            with tc.tile_pool(name="psSmall", bufs=4, space="PSUM") as psSm, \
                 tc.tile_pool(name="pt", bufs=18) as ptpool, \
                 tc.tile_pool(name="ynorm", bufs=3) as ypool, \
                 tc.tile_pool(name="recs", bufs=3) as recpool, \
                 tc.tile_pool(name="wo", bufs=3) as wopool, \
                 tc.tile_pool(name="osb", bufs=3) as opool:
                qk_stack = ExitStack()
                psProj = qk_stack.enter_context(
                    tc.tile_pool(name="psProj", bufs=2, space="PSUM"))
                pts = [[None] * TT for _ in range(HL)]
                ysbs = {}

                def qk_pair(h, kc):
                    kv = h // (HL // KVL)
                    pts[h][kc] = ptpool.tile([128, S], dtr, tag="pt",
                                             name=f"pt{h}_{kc}")
                    for tb in range(2):
                        sp = psSm.tile([128, 512], dt, tag="small", name="sp")
                        nc.tensor.matmul(
                            sp[:], kT[kv][:, kc * 128:(kc + 1) * 128],
                            qT[h][:, tb * 512:(tb + 1) * 512],
                            start=True, stop=True, skip_group_check=True)
                        nc.scalar.activation(
                            pts[h][kc][:, tb * 512:(tb + 1) * 512], sp[:],
                            mybir.ActivationFunctionType.Exp,
                            scale=float(SCALE))

                def pv_mm(h, qt):
                    kv = h // (HL // KVL)
                    yp = psSm.tile([128, 512], dt, tag="small", name="yp")
                    for kc in range(TT):
                        nc.tensor.matmul(
                            yp[:, 0:HD + 1],
                            pts[h][kc][:, qt * 128:(qt + 1) * 128],
                            v_nat[kv][kc][:],
                            start=(kc == 0), stop=(kc == TT - 1),
                            skip_group_check=True)
                    rec = recpool.tile([128, 1], dt, tag="rec", name="rec")
                    nc.vector.reciprocal(rec[:], yp[:, HD:HD + 1])
                    ysb = ypool.tile([128, HD], dtr, tag="ysb", name="ysb")
                    nc.vector.tensor_scalar_mul(ysb[:], yp[:, 0:HD], rec[:])
                    ysbs[(h, qt)] = ysb

                def pv_fin(h, qt):
                    ysb = ysbs.pop((h, qt))
                    ytp = psSm.tile([128, 128], dtr, tag="small", name="ytp")
                    nc.tensor.transpose(ytp[:], ysb[:], ident[:])
                    nc.vector.tensor_copy(yT[h][:, qt * 128:(qt + 1) * 128],
                                          ytp[:])

                extra_q = []

                def head_step(it, g, extra=None):
                    if g == 0:
                        fin_h, fin_qt = it - 3, 6
                        pv_h, pv_qt = it - 3, 7
                    elif g == 1:
                        fin_h, fin_qt = it - 3, 7
                        pv_h, pv_qt = it - 2, 0
                    else:
                        fin_h, fin_qt = it - 2, g - 2
                        pv_h, pv_qt = it - 2, g - 1
                    # qk/pv matmuls run before each fin transpose so the DVE
                    # normalize chain it depends on is always covered
                    if g > 0 and 0 <= it - 1 < HL and g < TT:
                        qk_pair(it - 1, g)
                    if extra is not None:
                        extra()
                    for _ in range(4 if it == 1 else 2):
                        if extra_q:
                            extra_q.pop(0)()
                    if 0 <= pv_h < HL and pv_qt < TT:
                        pv_mm(pv_h, pv_qt)
                    if 0 <= fin_h < HL and (fin_h, fin_qt) in ysbs:
                        pv_fin(fin_h, fin_qt)
                    if g == 0 and 0 <= it - 1 < HL:
                        qk_pair(it - 1, 0)

                def head_chunk(it, wt):
                    # tb-outer so the first half's rope overlaps the second
                    # half's matmuls, shortening the qT critical chain
                    acc = psProj.tile([128, S], dt, tag="acc", bufs=2,
                                      name="acc")
                    q = qpool.tile([128, S], dtr, tag="qT", name=f"qT{it}")
                    n = 0
                    for tb in range(2):
                        for ec in range(ECH):
                            nc.tensor.matmul(
                                acc[:, tb * 512:(tb + 1) * 512], wt[:, ec, :],
                                xs[:, ec, tb * 512:(tb + 1) * 512],
                                start=(ec == 0), stop=(ec == ECH - 1),
                                skip_group_check=True)
                            n += 1
                            if it >= 1 and n % 8 == 0:
                                head_step(it, n // 8 - 1)
                        if tb == 0 and it + 1 < HL:
                            wts[it + 1] = w_dma(it + 1, f"wt_q{it + 1}")
                        rope_half(q, acc, tb)
                    return q

                def v_unit(i, kt):
                    pt = psSm.tile([128, 128], dtr, tag="small", name="vtp")
                    nc.tensor.transpose(
                        pt[:], vtmps[i][:, kt * 128:(kt + 1) * 128], ident[:])
                    nc.vector.tensor_copy(v_nat[i][kt][:, 0:HD], pt[:])
                    nc.vector.memset(v_nat[i][kt][:, HD:HD + 1], 1.0)

                # V transposes: half fill the PE at the pool boundary,
                # half cover iteration 1's exposed rope wait
                for kt in range(TT):
                    v_unit(0, kt)
                extra_q.extend(lambda kt=kt: v_unit(1, kt)
                               for kt in range(TT))
                for it in range(HL):
                    qT[it] = head_chunk(it, wts[it])
                # virtual iteration 8 drains QK of head 7 + PV of heads 5/6
                for g in range(TT):
                    head_step(HL, g)
                qk_stack.close()  # free psProj banks for psO

                def wo_dma(oc):
                    wt = wopool.tile([128, HL, 128], dtr, tag="wo",
                                     name=f"wt_o{oc}")
                    nc.sync.dma_start(
                        out=wt[:],
                        in_=wo_d[oc * 128:(oc + 1) * 128, :].rearrange(
                            "p (c m) -> p c m", m=128))
                    return wt

                def e_half(op, wt, oc, tb, yc_list, start, stop, ot=None):
                    for yc in yc_list:
                        nc.tensor.matmul(
                            op[:, tb * 512:(tb + 1) * 512], wt[:, yc, :],
                            yT[yc][:, tb * 512:(tb + 1) * 512],
                            start=(start and yc == yc_list[0]),
                            stop=(stop and yc == yc_list[-1]),
                            skip_group_check=True)
                    if ot is not None:
                        nc.scalar.copy(ot[:, tb * 512:(tb + 1) * 512],
                                       op[:, tb * 512:(tb + 1) * 512])
                        nc.sync.dma_start(
                            out=out_d[oc * 128:(oc + 1) * 128,
                                      tb * 512:(tb + 1) * 512],
                            in_=ot[:, tb * 512:(tb + 1) * 512])

                # ------------ out projection (partial, transposed, fp16) ----
                # oc 0/1 accumulate heads 0-5 interleaved into the PV drain of
                # heads 6/7, so the tail never idles the PE
                with tc.tile_pool(name="psO", bufs=2, space="PSUM") as psO:
                    wt_o01 = [wo_dma(0), wo_dma(1)]
                    op01 = [psO.tile([128, S], dt, tag="op", name=f"op{j}")
                            for j in range(2)]
                    ethunks = []
                    for j in range(2):
                        for tb in range(2):
                            for y0 in (0, 2, 4):
                                ethunks.append(
                                    lambda j=j, tb=tb, y0=y0: e_half(
                                        op01[j], wt_o01[j], j, tb,
                                        [y0, y0 + 1], start=(y0 == 0),
                                        stop=False))
                    # yc=6 contributions become legal once head 6 finishes
                    # (virtual iteration 9, group 1) — keep them last
                    for j in range(2):
                        for tb in range(2):
                            ethunks.append(
                                lambda j=j, tb=tb: e_half(
                                    op01[j], wt_o01[j], j, tb, [6],
                                    start=False, stop=False))

                    def extra2():
                        for _ in range(2):
                            if ethunks:
                                ethunks.pop(0)()

                    for g in range(TT):
                        head_step(HL + 1, g, extra=extra2)
                    head_step(HL + 2, 0, extra=extra2)
                    head_step(HL + 2, 1, extra=extra2)
                    while ethunks:
                        ethunks.pop(0)()
                    for j in range(2):
                        ot = opool.tile([128, S], dtr, tag="ot", name="ot")
                        for tb in range(2):
                            e_half(op01[j], wt_o01[j], j, tb, [7],
                                   start=False, stop=True, ot=ot)
                    for oc in range(2, E // 128):
                        wt = wo_dma(oc)
                        op = psO.tile([128, S], dt, tag="op", name="op")
                        ot = opool.tile([128, S], dtr, tag="ot", name="ot")
                        for tb in range(2):
                            e_half(op, wt, oc, tb, list(range(HL)),
                                   start=True, stop=True, ot=ot)

    nc.compile()
    return nc


def _rope_tables():
    inv = 1.0 / (10000.0 ** (np.arange(0, HD, 2, dtype=np.float32) / HD))  # [64]
    ang = np.arange(S, dtype=np.float32)[None, :] * inv[:, None]           # [64, S]
    cos = np.concatenate([np.cos(ang), np.cos(ang)], axis=0).astype(np.float32)   # [128, S]
    sin = np.sin(ang)
    sinp = np.concatenate([-sin, sin], axis=0).astype(np.float32)          # [128, S]
    return cos, sinp


def _rearrange_w(w, n_chunks):
    # [E_rows, n_chunks*128] -> [n_chunks*128, E_rows] blocks: row cc*128+p
    # holds w[c*128+p, cc*128+m] at col c*128+m
    e_rows = w.shape[0]
    c = e_rows // 128
    return np.ascontiguousarray(
        w.reshape(c, 128, n_chunks, 128).transpose(2, 1, 0, 3).reshape(
            n_chunks * 128, e_rows))


def make_in_maps(x, wq, wk, wv, wo):
    cos, sinp = _rope_tables()
    ndt = np.float16 if MM_DT == "float16" else np.float32
    x = np.ascontiguousarray(x, dtype=np.float32)
    in_maps = []
    for c in range(N_CORES):
        b, r = c // TP, c % TP
        in_maps.append({
            "xt": np.ascontiguousarray(x[b].T).astype(ndt),
            "wq": _rearrange_w(
                wq[:, r * QCOLS:(r + 1) * QCOLS].astype(ndt), HL),
            "wk": _rearrange_w(
                wk[:, r * KVCOLS:(r + 1) * KVCOLS].astype(ndt), KVL),
            "wv": _rearrange_w(
                wv[:, r * KVCOLS:(r + 1) * KVCOLS].astype(ndt), KVL),
            "wo": _rearrange_w(
                wo[r * QCOLS:(r + 1) * QCOLS, :].astype(ndt), ECH),
            "cos": cos.astype(ndt),
            "sinp": sinp.astype(ndt),
        })
    return in_maps


def kernel(x, wq, wk, wv, wo):
    global _PROGRAM
    from concourse.bass_utils import run_bass_kernel_spmd

    if _PROGRAM is None:
        _PROGRAM = _build_program()
    nc = _PROGRAM

    res = run_bass_kernel_spmd(nc, make_in_maps(x, wq, wk, wv, wo),
                               list(range(N_CORES)))

    out = np.zeros((B, S, E), dtype=np.float32)
    for c in range(N_CORES):
        b = c // TP
        out[b] += res.results[c]["out_t"].T.astype(np.float32)
    return out


# revision 39
# speedup vs baseline: 1.0411x; 1.0081x over previous
"""GQA (B=2,S=1024,E=4096,H=32,KV=8,HD=128, RoPE, no causal mask) on 8 NeuronCores.

Sharding: 2 batch-groups x 4-way head tensor-parallel.
Core c: batch b=c//4, tp rank r=c%4 -> 8 q heads [8r,8r+8), 2 kv heads [2r,2r+2),
wo rows [1024r, 1024(r+1)).  Each core computes a partial output
out_part = y_local @ wo[local_rows, :]  (emitted transposed as [4096, 1024] fp16);
host sums the 4 partials per batch. No device collectives needed.

v4: single fused pipeline.
- Projections are chunk-major (full-E accumulation in PSUM), order
  K0,K1,V0 interleaved per e-chunk (tracks the x DMA stream), V1, Q0..Q7.
- Head h's QK+exp / PV / y-transpose are slot-scheduled into chunk h+1 / h+2's
  projection groups so scalar-engine exp (~110us) and all DVE chains hide
  under Tensor work.
- Weights are host-prearranged so every weight DMA is contiguous per
  partition; output DMA is fp16 (host accumulates partials in fp32).
"""
import sys

sys.path.insert(0, "/opt/trn_rl_repo")

import numpy as np

B = 2
S = 1024
E = 4096
HD = 128
N_CORES = 8
TP = 4            # tensor-parallel ranks per batch group
HL = 8            # q heads per core
KVL = 2           # kv heads per core
QCOLS = HL * HD   # 1024
KVCOLS = KVL * HD  # 256
ECH = E // 128    # 32 e-chunks
TT = S // 128     # 8 token tiles
SCALE = 1.0 / np.sqrt(np.float32(HD))
MM_DT = "float16"

_PROGRAM = None


def _build_program():
    import concourse.bass as bass  # noqa: F401
    from concourse import bacc
    import concourse.mybir as mybir
    from concourse.tile import TileContext
    from concourse.masks import make_identity

    dt = mybir.dt.float32
    dtr = getattr(mybir.dt, MM_DT)
    nc = bacc.Bacc("TRN2", target_bir_lowering=False, debug=False,
                   num_devices=N_CORES)

    xt_d = nc.declare_dram_parameter("xt", [E, S], dtr, isOutput=False)
    # host-prearranged: row block cc*128+p holds w[:, cc*128:...] row c*128+p
    wq_d = nc.declare_dram_parameter("wq", [HL * 128, E], dtr, isOutput=False)
    wk_d = nc.declare_dram_parameter("wk", [KVL * 128, E], dtr, isOutput=False)
    wv_d = nc.declare_dram_parameter("wv", [KVL * 128, E], dtr, isOutput=False)
    wo_d = nc.declare_dram_parameter("wo", [ECH * 128, QCOLS], dtr,
                                     isOutput=False)
    cos_d = nc.declare_dram_parameter("cos", [HD, S], dtr, isOutput=False)
    sinp_d = nc.declare_dram_parameter("sinp", [HD, S], dtr, isOutput=False)
    out_d = nc.declare_dram_parameter("out_t", [E, S], dtr, isOutput=True)

    def w_src(cc):
        # [128, ECH, 128] view of chunk cc's weights, contiguous per partition
        if cc < HL:
            base = wq_d
        elif cc < HL + KVL:
            base, cc = wk_d, cc - HL
        else:
            base, cc = wv_d, cc - HL - KVL
        return base[cc * 128:(cc + 1) * 128, :].rearrange(
            "p (c m) -> p c m", m=128)

    with TileContext(nc) as tc:
        with tc.tile_pool(name="const", bufs=1) as cpool, \
             tc.tile_pool(name="persist", bufs=1) as ppool, \
             tc.tile_pool(name="vnat", bufs=1) as vpool, \
             tc.tile_pool(name="wstream", bufs=5) as wpool, \
             tc.tile_pool(name="qroll", bufs=3) as qpool, \
             tc.tile_pool(name="rope", bufs=2) as ropool:
            ident_f = cpool.tile([128, 128], dt)
            make_identity(nc, ident_f[:])
            ident = cpool.tile([128, 128], dtr)
            nc.scalar.copy(ident[:], ident_f[:])
            cos_t = cpool.tile([HD, S], dtr, tag="cos")
            sinp_t = cpool.tile([HD, S], dtr, tag="sinp")

            # persistent data
            xs = ppool.tile([128, ECH, S], dtr, tag="xs", name="xs")
            kT = [ppool.tile([128, S], dtr, tag=f"kT{i}", name=f"kT{i}")
                  for i in range(KVL)]
            yT = [ppool.tile([128, S], dtr, tag=f"yT{i}", name=f"yT{i}")
                  for i in range(HL)]
            v_nat = [[vpool.tile([128, HD + 1], dtr, tag=f"v{kv}_{kt}",
                                 name=f"v{kv}_{kt}")
                      for kt in range(TT)] for kv in range(KVL)]

            # DMA emission order matters: the Sync engine issues descriptors
            # in order at ~240-330GB/s aggregate, so stage the first four
            # chunks' weights per-superchunk between x slices.
            pre_cc = [HL, HL + 1, HL + KVL, HL + KVL + 1]  # K0, K1, V0, V1
            pre_wt = [wpool.tile([128, ECH, 128], dtr, tag="w",
                                 name=f"wt_pre{j}") for j in range(4)]
            for es in range(4):
                if es == 0:  # alternate weight quarters and x slices so the
                    # first few matmuls of every chunk have data earliest
                    for j in range(4):
                        nc.sync.dma_start(out=pre_wt[j][:, 0:8, :],
                                          in_=w_src(pre_cc[j])[:, 0:8, :])
                        nc.sync.dma_start(out=xs[:, j, :],
                                          in_=xt_d[j * 128:(j + 1) * 128, :])
                    ecs = range(4, 8)
                else:
                    for j in range(4):
                        nc.sync.dma_start(
                            out=pre_wt[j][:, es * 8:(es + 1) * 8, :],
                            in_=w_src(pre_cc[j])[:, es * 8:(es + 1) * 8, :])
                    ecs = range(es * 8, (es + 1) * 8)
                for ec in ecs:
                    nc.sync.dma_start(out=xs[:, ec, :],
                                      in_=xt_d[ec * 128:(ec + 1) * 128, :])
                if es == 1:
                    nc.sync.dma_start(out=cos_t[:], in_=cos_d[:])
                    nc.sync.dma_start(out=sinp_t[:], in_=sinp_d[:])

            def w_dma(cc, name):
                wt = wpool.tile([128, ECH, 128], dtr, tag="w", name=name)
                nc.sync.dma_start(out=wt[:], in_=w_src(cc)[:])
                return wt

            def rope_half(dstT, acc, tb):
                lo, hi = tb * 512, (tb + 1) * 512
                tmp = ropool.tile([HD, 512], dtr, tag=f"t0{tb}", name="tmp")
                nc.scalar.copy(tmp[:], acc[:, lo:hi])
                sh = ropool.tile([HD, 512], dtr, tag=f"sh{tb}", name="sh")
                nc.sync.dma_start(out=sh[0:64, :], in_=tmp[64:128, :])
                nc.sync.dma_start(out=sh[64:128, :], in_=tmp[0:64, :])
                t1 = ropool.tile([HD, 512], dtr, tag=f"t1{tb}", name="t1")
                nc.vector.tensor_mul(t1[:], tmp[:], cos_t[:, lo:hi])
                nc.vector.tensor_mul(sh[:], sh[:], sinp_t[:, lo:hi])
                nc.vector.tensor_add(dstT[:, lo:hi], t1[:], sh[:])

            # ---------------- pre-head phase ----------------
            # all four K/V chunks interleaved per e-chunk so compute tracks
            # the x DMA stream (4 accumulators = all 8 PSUM banks)
            vtmps = []
            qT = [None] * HL
            wts = [None] * (HL + 1)
            with tc.tile_pool(name="psPre", bufs=1, space="PSUM") as psPre:
                pre_acc = [psPre.tile([128, S], dt, tag="acc", bufs=4,
                                      name=f"accp{j}") for j in range(4)]

                def pre_mm(j, ec, stop):
                    for tb in range(2):
                        nc.tensor.matmul(
                            pre_acc[j][:, tb * 512:(tb + 1) * 512],
                            pre_wt[j][:, ec, :],
                            xs[:, ec, tb * 512:(tb + 1) * 512],
                            start=(ec == 0), stop=stop,
                            skip_group_check=True)

                for ec in range(ECH - 4):
                    for j in range(4):
                        pre_mm(j, ec, stop=False)
                wts[0] = w_dma(0, "wt_q0")
                # K chunks finish first: their rope copies drain this pool's
                # banks while the V chunks' tail matmuls still run
                for i in range(KVL):
                    for ec in range(ECH - 4, ECH):
                        pre_mm(i, ec, stop=(ec == ECH - 1))
                    rope_half(kT[i], pre_acc[i], 0)
                    rope_half(kT[i], pre_acc[i], 1)
                for i in range(KVL):
                    for ec in range(ECH - 4, ECH):
                        pre_mm(2 + i, ec, stop=(ec == ECH - 1))
                    vtmp = ropool.tile([128, S], dtr, tag=f"vt{i}",
                                       name="vtmp", bufs=1)
                    nc.vector.tensor_copy(vtmp[:, 0:512],
                                          pre_acc[2 + i][:, 0:512])
                    nc.vector.tensor_copy(vtmp[:, 512:S],
                                          pre_acc[2 + i][:, 512:S])
                    vtmps.append(vtmp)

            # ---------------- head loop ----------------
            # slot schedule inside chunk `it`'s projection, group g (0..7):
            #   g=0:  fin(it-3, 6), pv(it-3, 7), qk(it-1, 0)
            #   g=1:  fin(it-3, 7), pv(it-2, 0), qk(it-1, 1)
            #   g>=2: fin(it-2, g-2), pv(it-2, g-1), qk(it-1, g)
            # so every transpose (fin) trails its PV block by two groups and
            # exp for head it-1 is paced across the whole chunk.
            from contextlib import ExitStack
            with tc.tile_pool(name="psSmall", bufs=4, space="PSUM") as psSm, \
                 tc.tile_pool(name="pt", bufs=18) as ptpool, \
                 tc.tile_pool(name="ynorm", bufs=3) as ypool, \
                 tc.tile_pool(name="recs", bufs=3) as recpool, \
                 tc.tile_pool(name="wo", bufs=3) as wopool, \
                 tc.tile_pool(name="osb", bufs=3) as opool:
                qk_stack = ExitStack()
                psProj = qk_stack.enter_context(
                    tc.tile_pool(name="psProj", bufs=2, space="PSUM"))
                pts = [[None] * TT for _ in range(HL)]
                ysbs = {}

                def qk_pair(h, kc):
                    kv = h // (HL // KVL)
                    pts[h][kc] = ptpool.tile([128, S], dtr, tag="pt",
                                             name=f"pt{h}_{kc}")
                    for tb in range(2):
                        sp = psSm.tile([128, 512], dt, tag="small", name="sp")
                        nc.tensor.matmul(
                            sp[:], kT[kv][:, kc * 128:(kc + 1) * 128],
                            qT[h][:, tb * 512:(tb + 1) * 512],
                            start=True, stop=True, skip_group_check=True)
                        nc.scalar.activation(
                            pts[h][kc][:, tb * 512:(tb + 1) * 512], sp[:],
                            mybir.ActivationFunctionType.Exp,
                            scale=float(SCALE))

                def pv_mm(h, qt):
                    kv = h // (HL // KVL)
                    yp = psSm.tile([128, 512], dt, tag="small", name="yp")
                    for kc in range(TT):
                        nc.tensor.matmul(
                            yp[:, 0:HD + 1],
                            pts[h][kc][:, qt * 128:(qt + 1) * 128],
                            v_nat[kv][kc][:],
                            start=(kc == 0), stop=(kc == TT - 1),
                            skip_group_check=True)
                    rec = recpool.tile([128, 1], dt, tag="rec", name="rec")
                    nc.vector.reciprocal(rec[:], yp[:, HD:HD + 1])
                    ysb = ypool.tile([128, HD], dtr, tag="ysb", name="ysb")
                    nc.vector.tensor_scalar_mul(ysb[:], yp[:, 0:HD], rec[:])
                    ysbs[(h, qt)] = ysb

                def pv_fin(h, qt):
                    ysb = ysbs.pop((h, qt))
                    ytp = psSm.tile([128, 128], dtr, tag="small", name="ytp")
                    nc.tensor.transpose(ytp[:], ysb[:], ident[:])
                    nc.vector.tensor_copy(yT[h][:, qt * 128:(qt + 1) * 128],
                                          ytp[:])

                extra_q = []

                def head_step(it, g, extra=None):
                    if g == 0:
                        fin_h, fin_qt = it - 3, 6
                        pv_h, pv_qt = it - 3, 7
                    elif g == 1:
                        fin_h, fin_qt = it - 3, 7
                        pv_h, pv_qt = it - 2, 0
                    else:
                        fin_h, fin_qt = it - 2, g - 2
                        pv_h, pv_qt = it - 2, g - 1
                    # qk/pv matmuls run before each fin transpose so the DVE
                    # normalize chain it depends on is always covered
                    if g > 0 and 0 <= it - 1 < HL and g < TT:
                        qk_pair(it - 1, g - 1 if it == 1 else g)
                        if it == 1 and g == TT - 1:
                            qk_pair(0, TT - 1)
                    if extra is not None:
                        extra()
                    for _ in range(4 if it == 1 else 2):
                        if extra_q:
                            extra_q.pop(0)()
                    if 0 <= pv_h < HL and pv_qt < TT:
                        pv_mm(pv_h, pv_qt)
                    if 0 <= fin_h < HL and (fin_h, fin_qt) in ysbs:
                        pv_fin(fin_h, fin_qt)
                    if g == 0 and 0 <= it - 1 < HL and it != 1:
                        qk_pair(it - 1, 0)

                def head_chunk(it, wt):
                    # tb-outer so the first half's rope overlaps the second
                    # half's matmuls, shortening the qT critical chain
                    acc = psProj.tile([128, S], dt, tag="acc", bufs=2,
                                      name="acc")
                    q = qpool.tile([128, S], dtr, tag="qT", name=f"qT{it}")
                    n = 0
                    for tb in range(2):
                        for ec in range(ECH):
                            nc.tensor.matmul(
                                acc[:, tb * 512:(tb + 1) * 512], wt[:, ec, :],
                                xs[:, ec, tb * 512:(tb + 1) * 512],
                                start=(ec == 0), stop=(ec == ECH - 1),
                                skip_group_check=True)
                            n += 1
                            if it >= 1 and n % 8 == 0:
                                head_step(it, n // 8 - 1)
                        if tb == 0 and it + 1 < HL:
                            wts[it + 1] = w_dma(it + 1, f"wt_q{it + 1}")
                        rope_half(q, acc, tb)
                    return q

                def v_unit(i, kt):
                    pt = psSm.tile([128, 128], dtr, tag="small", name="vtp")
                    nc.tensor.transpose(
                        pt[:], vtmps[i][:, kt * 128:(kt + 1) * 128], ident[:])
                    nc.vector.tensor_copy(v_nat[i][kt][:, 0:HD], pt[:])
                    nc.vector.memset(v_nat[i][kt][:, HD:HD + 1], 1.0)

                # V transposes: half fill the PE at the pool boundary,
                # half cover iteration 1's exposed rope wait
                for kt in range(TT):
                    v_unit(0, kt)
                extra_q.extend(lambda kt=kt: v_unit(1, kt)
                               for kt in range(TT))
                for it in range(HL):
                    qT[it] = head_chunk(it, wts[it])
                # virtual iteration 8 drains QK of head 7 + PV of heads 5/6
                for g in range(TT):
                    head_step(HL, g)
                qk_stack.close()  # free psProj banks for psO

                def wo_dma(oc):
                    wt = wopool.tile([128, HL, 128], dtr, tag="wo",
                                     name=f"wt_o{oc}")
                    nc.sync.dma_start(
                        out=wt[:],
                        in_=wo_d[oc * 128:(oc + 1) * 128, :].rearrange(
                            "p (c m) -> p c m", m=128))
                    return wt

                def e_half(op, wt, oc, tb, yc_list, start, stop, ot=None):
                    for yc in yc_list:
                        nc.tensor.matmul(
                            op[:, tb * 512:(tb + 1) * 512], wt[:, yc, :],
                            yT[yc][:, tb * 512:(tb + 1) * 512],
                            start=(start and yc == yc_list[0]),
                            stop=(stop and yc == yc_list[-1]),
                            skip_group_check=True)
                    if ot is not None:
                        nc.scalar.copy(ot[:, tb * 512:(tb + 1) * 512],
                                       op[:, tb * 512:(tb + 1) * 512])
                        nc.sync.dma_start(
                            out=out_d[oc * 128:(oc + 1) * 128,
                                      tb * 512:(tb + 1) * 512],
                            in_=ot[:, tb * 512:(tb + 1) * 512])

                # ------------ out projection (partial, transposed, fp16) ----
                # oc 0/1 accumulate heads 0-5 interleaved into the PV drain of
                # heads 6/7, so the tail never idles the PE
                with tc.tile_pool(name="psO", bufs=2, space="PSUM") as psO:
                    wt_o01 = [wo_dma(0), wo_dma(1)]
                    op01 = [psO.tile([128, S], dt, tag="op", name=f"op{j}")
                            for j in range(2)]
                    ethunks = []
                    for j in range(2):
                        for tb in range(2):
                            for y0 in (0, 2, 4):
                                ethunks.append(
                                    lambda j=j, tb=tb, y0=y0: e_half(
                                        op01[j], wt_o01[j], j, tb,
                                        [y0, y0 + 1], start=(y0 == 0),
                                        stop=False))
                    # yc=6 contributions become legal once head 6 finishes
                    # (virtual iteration 9, group 1) — keep them last
                    for j in range(2):
                        for tb in range(2):
                            ethunks.append(
                                lambda j=j, tb=tb: e_half(
                                    op01[j], wt_o01[j], j, tb, [6],
                                    start=False, stop=False))

                    def extra2():
                        for _ in range(2):
                            if ethunks:
                                ethunks.pop(0)()

                    for g in range(TT):
                        head_step(HL + 1, g, extra=extra2)
                    head_step(HL + 2, 0, extra=extra2)
                    head_step(HL + 2, 1, extra=extra2)
                    while ethunks:
                        ethunks.pop(0)()
                    for j in range(2):
                        ot = opool.tile([128, S], dtr, tag="ot", name="ot")
                        for tb in range(2):
                            e_half(op01[j], wt_o01[j], j, tb, [7],
                                   start=False, stop=True, ot=ot)
                    for oc in range(2, E // 128):
                        wt = wo_dma(oc)
                        op = psO.tile([128, S], dt, tag="op", name="op")
                        ot = opool.tile([128, S], dtr, tag="ot", name="ot")
                        for tb in range(2):
                            e_half(op, wt, oc, tb, list(range(HL)),
                                   start=True, stop=True, ot=ot)

    nc.compile()
    return nc


def _rope_tables():
    inv = 1.0 / (10000.0 ** (np.arange(0, HD, 2, dtype=np.float32) / HD))  # [64]
    ang = np.arange(S, dtype=np.float32)[None, :] * inv[:, None]           # [64, S]
    cos = np.concatenate([np.cos(ang), np.cos(ang)], axis=0).astype(np.float32)   # [128, S]
    sin = np.sin(ang)
    sinp = np.concatenate([-sin, sin], axis=0).astype(np.float32)          # [128, S]
    return cos, sinp


def _rearrange_w(w, n_chunks):
    # [E_rows, n_chunks*128] -> [n_chunks*128, E_rows] blocks: row cc*128+p
    # holds w[c*128+p, cc*128+m] at col c*128+m
    e_rows = w.shape[0]
    c = e_rows // 128
    return np.ascontiguousarray(
        w.reshape(c, 128, n_chunks, 128).transpose(2, 1, 0, 3).reshape(
            n_chunks * 128, e_rows))


def make_in_maps(x, wq, wk, wv, wo):
    cos, sinp = _rope_tables()
    ndt = np.float16 if MM_DT == "float16" else np.float32
    x = np.ascontiguousarray(x, dtype=np.float32)
    in_maps = []
    for c in range(N_CORES):
        b, r = c // TP, c % TP
        in_maps.append({
            "xt": np.ascontiguousarray(x[b].T).astype(ndt),
            "wq": _rearrange_w(
                wq[:, r * QCOLS:(r + 1) * QCOLS].astype(ndt), HL),
            "wk": _rearrange_w(
                wk[:, r * KVCOLS:(r + 1) * KVCOLS].astype(ndt), KVL),
            "wv": _rearrange_w(
                wv[:, r * KVCOLS:(r + 1) * KVCOLS].astype(ndt), KVL),
            "wo": _rearrange_w(
                wo[r * QCOLS:(r + 1) * QCOLS, :].astype(ndt), ECH),
            "cos": cos.astype(ndt),
            "sinp": sinp.astype(ndt),
        })
    return in_maps


def kernel(x, wq, wk, wv, wo):
    global _PROGRAM
    from concourse.bass_utils import run_bass_kernel_spmd

    if _PROGRAM is None:
        _PROGRAM = _build_program()
    nc = _PROGRAM

    res = run_bass_kernel_spmd(nc, make_in_maps(x, wq, wk, wv, wo),
                               list(range(N_CORES)))

    out = np.zeros((B, S, E), dtype=np.float32)
    for c in range(N_CORES):
        b = c // TP
        out[b] += res.results[c]["out_t"].T.astype(np.float32)
    return out


# revision 40
# speedup vs baseline: 1.0443x; 1.0031x over previous
"""GQA (B=2,S=1024,E=4096,H=32,KV=8,HD=128, RoPE, no causal mask) on 8 NeuronCores.

Sharding: 2 batch-groups x 4-way head tensor-parallel.
Core c: batch b=c//4, tp rank r=c%4 -> 8 q heads [8r,8r+8), 2 kv heads [2r,2r+2),
wo rows [1024r, 1024(r+1)).  Each core computes a partial output
out_part = y_local @ wo[local_rows, :]  (emitted transposed as [4096, 1024] fp16);
host sums the 4 partials per batch. No device collectives needed.

v4: single fused pipeline.
- Projections are chunk-major (full-E accumulation in PSUM), order
  K0,K1,V0 interleaved per e-chunk (tracks the x DMA stream), V1, Q0..Q7.
- Head h's QK+exp / PV / y-transpose are slot-scheduled into chunk h+1 / h+2's
  projection groups so scalar-engine exp (~110us) and all DVE chains hide
  under Tensor work.
- Weights are host-prearranged so every weight DMA is contiguous per
  partition; output DMA is fp16 (host accumulates partials in fp32).
"""
import sys

sys.path.insert(0, "/opt/trn_rl_repo")

import numpy as np

B = 2
S = 1024
E = 4096
HD = 128
N_CORES = 8
TP = 4            # tensor-parallel ranks per batch group
HL = 8            # q heads per core
KVL = 2           # kv heads per core
QCOLS = HL * HD   # 1024
KVCOLS = KVL * HD  # 256
ECH = E // 128    # 32 e-chunks
TT = S // 128     # 8 token tiles
SCALE = 1.0 / np.sqrt(np.float32(HD))
MM_DT = "float16"

_PROGRAM = None


def _build_program():
    import concourse.bass as bass  # noqa: F401
    from concourse import bacc
    import concourse.mybir as mybir
    from concourse.tile import TileContext
    from concourse.masks import make_identity

    dt = mybir.dt.float32
    dtr = getattr(mybir.dt, MM_DT)
    nc = bacc.Bacc("TRN2", target_bir_lowering=False, debug=False,
                   num_devices=N_CORES)

    xt_d = nc.declare_dram_parameter("xt", [E, S], dtr, isOutput=False)
    # host-prearranged: row block cc*128+p holds w[:, cc*128:...] row c*128+p
    wq_d = nc.declare_dram_parameter("wq", [HL * 128, E], dtr, isOutput=False)
    wk_d = nc.declare_dram_parameter("wk", [KVL * 128, E], dtr, isOutput=False)
    wv_d = nc.declare_dram_parameter("wv", [KVL * 128, E], dtr, isOutput=False)
    wo_d = nc.declare_dram_parameter("wo", [ECH * 128, QCOLS], dtr,
                                     isOutput=False)
    cos_d = nc.declare_dram_parameter("cos", [HD, S], dtr, isOutput=False)
    sinp_d = nc.declare_dram_parameter("sinp", [HD, S], dtr, isOutput=False)
    out_d = nc.declare_dram_parameter("out_t", [E, S], dtr, isOutput=True)

    def w_src(cc):
        # [128, ECH, 128] view of chunk cc's weights, contiguous per partition
        if cc < HL:
            base = wq_d
        elif cc < HL + KVL:
            base, cc = wk_d, cc - HL
        else:
            base, cc = wv_d, cc - HL - KVL
        return base[cc * 128:(cc + 1) * 128, :].rearrange(
            "p (c m) -> p c m", m=128)

    with TileContext(nc) as tc:
        with tc.tile_pool(name="const", bufs=1) as cpool, \
             tc.tile_pool(name="persist", bufs=1) as ppool, \
             tc.tile_pool(name="vnat", bufs=1) as vpool, \
             tc.tile_pool(name="wstream", bufs=5) as wpool, \
             tc.tile_pool(name="qroll", bufs=3) as qpool, \
             tc.tile_pool(name="rope", bufs=2) as ropool:
            ident_f = cpool.tile([128, 128], dt)
            make_identity(nc, ident_f[:])
            ident = cpool.tile([128, 128], dtr)
            nc.scalar.copy(ident[:], ident_f[:])
            cos_t = cpool.tile([HD, S], dtr, tag="cos")
            sinp_t = cpool.tile([HD, S], dtr, tag="sinp")

            # persistent data
            xs = ppool.tile([128, ECH, S], dtr, tag="xs", name="xs")
            kT = [ppool.tile([128, S], dtr, tag=f"kT{i}", name=f"kT{i}")
                  for i in range(KVL)]
            yT = [ppool.tile([128, S], dtr, tag=f"yT{i}", name=f"yT{i}")
                  for i in range(HL)]
            v_nat = [[vpool.tile([128, HD + 1], dtr, tag=f"v{kv}_{kt}",
                                 name=f"v{kv}_{kt}")
                      for kt in range(TT)] for kv in range(KVL)]

            # DMA emission order matters: the Sync engine issues descriptors
            # in order at ~240-330GB/s aggregate, so stage the first four
            # chunks' weights per-superchunk between x slices.
            pre_cc = [HL, HL + 1, HL + KVL, HL + KVL + 1]  # K0, K1, V0, V1
            pre_wt = [wpool.tile([128, ECH, 128], dtr, tag="w",
                                 name=f"wt_pre{j}") for j in range(4)]
            for es in range(4):
                if es == 0:  # alternate weight quarters and x slices so the
                    # first few matmuls of every chunk have data earliest
                    for j in range(4):
                        nc.sync.dma_start(out=pre_wt[j][:, 0:8, :],
                                          in_=w_src(pre_cc[j])[:, 0:8, :])
                        nc.sync.dma_start(out=xs[:, j, :],
                                          in_=xt_d[j * 128:(j + 1) * 128, :])
                    ecs = range(4, 8)
                else:
                    for j in range(4):
                        nc.sync.dma_start(
                            out=pre_wt[j][:, es * 8:(es + 1) * 8, :],
                            in_=w_src(pre_cc[j])[:, es * 8:(es + 1) * 8, :])
                    ecs = range(es * 8, (es + 1) * 8)
                for ec in ecs:
                    nc.sync.dma_start(out=xs[:, ec, :],
                                      in_=xt_d[ec * 128:(ec + 1) * 128, :])
                if es == 1:
                    nc.sync.dma_start(out=cos_t[:], in_=cos_d[:])
                    nc.sync.dma_start(out=sinp_t[:], in_=sinp_d[:])

            def w_dma(cc, name):
                wt = wpool.tile([128, ECH, 128], dtr, tag="w", name=name)
                nc.sync.dma_start(out=wt[:], in_=w_src(cc)[:])
                return wt

            def rope_half(dstT, acc, tb):
                lo, hi = tb * 512, (tb + 1) * 512
                tmp = ropool.tile([HD, 512], dtr, tag=f"t0{tb}", name="tmp")
                nc.scalar.copy(tmp[:], acc[:, lo:hi])
                sh = ropool.tile([HD, 512], dtr, tag=f"sh{tb}", name="sh")
                nc.sync.dma_start(out=sh[0:64, :], in_=tmp[64:128, :])
                nc.sync.dma_start(out=sh[64:128, :], in_=tmp[0:64, :])
                t1 = ropool.tile([HD, 512], dtr, tag=f"t1{tb}", name="t1")
                nc.vector.tensor_mul(t1[:], tmp[:], cos_t[:, lo:hi])
                nc.vector.tensor_mul(sh[:], sh[:], sinp_t[:, lo:hi])
                nc.vector.tensor_add(dstT[:, lo:hi], t1[:], sh[:])

            # ---------------- pre-head phase ----------------
            # all four K/V chunks interleaved per e-chunk so compute tracks
            # the x DMA stream (4 accumulators = all 8 PSUM banks)
            vtmps = []
            qT = [None] * HL
            wts = [None] * (HL + 1)
            with tc.tile_pool(name="psPre", bufs=1, space="PSUM") as psPre:
                pre_acc = [psPre.tile([128, S], dt, tag="acc", bufs=4,
                                      name=f"accp{j}") for j in range(4)]

                def pre_mm(j, ec, stop):
                    for tb in range(2):
                        nc.tensor.matmul(
                            pre_acc[j][:, tb * 512:(tb + 1) * 512],
                            pre_wt[j][:, ec, :],
                            xs[:, ec, tb * 512:(tb + 1) * 512],
                            start=(ec == 0), stop=stop,
                            skip_group_check=True)

                for ec in range(ECH - 4):
                    for j in range(4):
                        pre_mm(j, ec, stop=False)
                wts[0] = w_dma(0, "wt_q0")
                # K chunks finish first: their rope copies drain this pool's
                # banks while the V chunks' tail matmuls still run
                for i in range(KVL):
                    for ec in range(ECH - 4, ECH):
                        pre_mm(i, ec, stop=(ec == ECH - 1))
                    rope_half(kT[i], pre_acc[i], 0)
                    rope_half(kT[i], pre_acc[i], 1)
                for i in range(KVL):
                    for ec in range(ECH - 4, ECH):
                        pre_mm(2 + i, ec, stop=(ec == ECH - 1))
                    vtmp = ropool.tile([128, S], dtr, tag=f"vt{i}",
                                       name="vtmp", bufs=1)
                    nc.vector.tensor_copy(vtmp[:, 0:512],
                                          pre_acc[2 + i][:, 0:512])
                    nc.vector.tensor_copy(vtmp[:, 512:S],
                                          pre_acc[2 + i][:, 512:S])
                    vtmps.append(vtmp)

            # ---------------- head loop ----------------
            # slot schedule inside chunk `it`'s projection, group g (0..7):
            #   g=0:  fin(it-3, 6), pv(it-3, 7), qk(it-1, 0)
            #   g=1:  fin(it-3, 7), pv(it-2, 0), qk(it-1, 1)
            #   g>=2: fin(it-2, g-2), pv(it-2, g-1), qk(it-1, g)
            # so every transpose (fin) trails its PV block by two groups and
            # exp for head it-1 is paced across the whole chunk.
            from contextlib import ExitStack
            with tc.tile_pool(name="psSmall", bufs=4, space="PSUM") as psSm, \
                 tc.tile_pool(name="pt", bufs=18) as ptpool, \
                 tc.tile_pool(name="ynorm", bufs=3) as ypool, \
                 tc.tile_pool(name="recs", bufs=3) as recpool, \
                 tc.tile_pool(name="wo", bufs=3) as wopool, \
                 tc.tile_pool(name="osb", bufs=3) as opool:
                qk_stack = ExitStack()
                psProj = qk_stack.enter_context(
                    tc.tile_pool(name="psProj", bufs=2, space="PSUM"))
                pts = [[None] * TT for _ in range(HL)]
                ysbs = {}

                def qk_pair(h, kc):
                    kv = h // (HL // KVL)
                    pts[h][kc] = ptpool.tile([128, S], dtr, tag="pt",
                                             name=f"pt{h}_{kc}")
                    for tb in range(2):
                        sp = psSm.tile([128, 512], dt, tag="small", name="sp")
                        nc.tensor.matmul(
                            sp[:], kT[kv][:, kc * 128:(kc + 1) * 128],
                            qT[h][:, tb * 512:(tb + 1) * 512],
                            start=True, stop=True, skip_group_check=True)
                        nc.scalar.activation(
                            pts[h][kc][:, tb * 512:(tb + 1) * 512], sp[:],
                            mybir.ActivationFunctionType.Exp,
                            scale=float(SCALE))

                def pv_mm(h, qt):
                    kv = h // (HL // KVL)
                    yp = psSm.tile([128, 512], dt, tag="small", name="yp")
                    for kc in range(TT):
                        nc.tensor.matmul(
                            yp[:, 0:HD + 1],
                            pts[h][kc][:, qt * 128:(qt + 1) * 128],
                            v_nat[kv][kc][:],
                            start=(kc == 0), stop=(kc == TT - 1),
                            skip_group_check=True)
                    rec = recpool.tile([128, 1], dt, tag="rec", name="rec")
                    nc.vector.reciprocal(rec[:], yp[:, HD:HD + 1])
                    ysb = ypool.tile([128, HD], dtr, tag="ysb", name="ysb")
                    nc.vector.tensor_scalar_mul(ysb[:], yp[:, 0:HD], rec[:])
                    ysbs[(h, qt)] = ysb

                def pv_fin(h, qt):
                    ysb = ysbs.pop((h, qt))
                    ytp = psSm.tile([128, 128], dtr, tag="small", name="ytp")
                    nc.tensor.transpose(ytp[:], ysb[:], ident[:])
                    nc.vector.tensor_copy(yT[h][:, qt * 128:(qt + 1) * 128],
                                          ytp[:])

                extra_q = []

                def head_step(it, g, extra=None):
                    if g == 0:
                        fin_h, fin_qt = it - 3, 6
                        pv_h, pv_qt = it - 3, 7
                    elif g == 1:
                        fin_h, fin_qt = it - 3, 7
                        pv_h, pv_qt = it - 2, 0
                    else:
                        fin_h, fin_qt = it - 2, g - 2
                        pv_h, pv_qt = it - 2, g - 1
                    # qk/pv matmuls run before each fin transpose so the DVE
                    # normalize chain it depends on is always covered
                    if g > 0 and 0 <= it - 1 < HL and g < TT and it != HL:
                        qk_pair(it - 1, g - 1 if it == 1 else g)
                        if it == 1 and g == TT - 1:
                            qk_pair(0, TT - 1)
                    if extra is not None:
                        extra()
                    for _ in range(4 if it == 1 else 2):
                        if extra_q:
                            extra_q.pop(0)()
                    if 0 <= pv_h < HL and pv_qt < TT:
                        pv_mm(pv_h, pv_qt)
                    if 0 <= fin_h < HL and (fin_h, fin_qt) in ysbs:
                        pv_fin(fin_h, fin_qt)
                    if g == 0 and 0 <= it - 1 < HL and it not in (1, HL):
                        qk_pair(it - 1, 0)
                    if it == HL and g < TT:
                        # tail: qk last so PV/fin work covers the exp pacing
                        qk_pair(HL - 1, g)

                def head_chunk(it, wt):
                    # tb-outer so the first half's rope overlaps the second
                    # half's matmuls, shortening the qT critical chain
                    acc = psProj.tile([128, S], dt, tag="acc", bufs=2,
                                      name="acc")
                    q = qpool.tile([128, S], dtr, tag="qT", name=f"qT{it}")
                    n = 0
                    for tb in range(2):
                        for ec in range(ECH):
                            nc.tensor.matmul(
                                acc[:, tb * 512:(tb + 1) * 512], wt[:, ec, :],
                                xs[:, ec, tb * 512:(tb + 1) * 512],
                                start=(ec == 0), stop=(ec == ECH - 1),
                                skip_group_check=True)
                            n += 1
                            if it >= 1 and n % 8 == 0:
                                head_step(it, n // 8 - 1)
                        if tb == 0 and it + 1 < HL:
                            wts[it + 1] = w_dma(it + 1, f"wt_q{it + 1}")
                        rope_half(q, acc, tb)
                    return q

                def v_unit(i, kt):
                    pt = psSm.tile([128, 128], dtr, tag="small", name="vtp")
                    nc.tensor.transpose(
                        pt[:], vtmps[i][:, kt * 128:(kt + 1) * 128], ident[:])
                    nc.vector.tensor_copy(v_nat[i][kt][:, 0:HD], pt[:])
                    nc.vector.memset(v_nat[i][kt][:, HD:HD + 1], 1.0)

                # V transposes: half fill the PE at the pool boundary,
                # half cover iteration 1's exposed rope wait
                for kt in range(TT):
                    v_unit(0, kt)
                extra_q.extend(lambda kt=kt: v_unit(1, kt)
                               for kt in range(TT))
                for it in range(HL):
                    qT[it] = head_chunk(it, wts[it])
                # virtual iteration 8 drains QK of head 7 + PV of heads 5/6
                for g in range(TT):
                    head_step(HL, g)
                qk_stack.close()  # free psProj banks for psO

                def wo_dma(oc):
                    wt = wopool.tile([128, HL, 128], dtr, tag="wo",
                                     name=f"wt_o{oc}")
                    nc.sync.dma_start(
                        out=wt[:],
                        in_=wo_d[oc * 128:(oc + 1) * 128, :].rearrange(
                            "p (c m) -> p c m", m=128))
                    return wt

                def e_half(op, wt, oc, tb, yc_list, start, stop, ot=None):
                    for yc in yc_list:
                        nc.tensor.matmul(
                            op[:, tb * 512:(tb + 1) * 512], wt[:, yc, :],
                            yT[yc][:, tb * 512:(tb + 1) * 512],
                            start=(start and yc == yc_list[0]),
                            stop=(stop and yc == yc_list[-1]),
                            skip_group_check=True)
                    if ot is not None:
                        nc.scalar.copy(ot[:, tb * 512:(tb + 1) * 512],
                                       op[:, tb * 512:(tb + 1) * 512])
                        nc.sync.dma_start(
                            out=out_d[oc * 128:(oc + 1) * 128,
                                      tb * 512:(tb + 1) * 512],
                            in_=ot[:, tb * 512:(tb + 1) * 512])

                # ------------ out projection (partial, transposed, fp16) ----
                # oc 0/1 accumulate heads 0-5 interleaved into the PV drain of
                # heads 6/7, so the tail never idles the PE
                with tc.tile_pool(name="psO", bufs=2, space="PSUM") as psO:
                    wt_o01 = [wo_dma(0), wo_dma(1)]
                    op01 = [psO.tile([128, S], dt, tag="op", name=f"op{j}")
                            for j in range(2)]
                    ethunks = []
                    for j in range(2):
                        for tb in range(2):
                            for y0 in (0, 2, 4):
                                ethunks.append(
                                    lambda j=j, tb=tb, y0=y0: e_half(
                                        op01[j], wt_o01[j], j, tb,
                                        [y0, y0 + 1], start=(y0 == 0),
                                        stop=False))
                    # yc=6 contributions become legal once head 6 finishes
                    # (virtual iteration 9, group 1) — keep them last
                    for j in range(2):
                        for tb in range(2):
                            ethunks.append(
                                lambda j=j, tb=tb: e_half(
                                    op01[j], wt_o01[j], j, tb, [6],
                                    start=False, stop=False))

                    def extra2():
                        for _ in range(2):
                            if ethunks:
                                ethunks.pop(0)()

                    for g in range(TT):
                        head_step(HL + 1, g, extra=extra2)
                    head_step(HL + 2, 0, extra=extra2)
                    head_step(HL + 2, 1, extra=extra2)
                    while ethunks:
                        ethunks.pop(0)()
                    for j in range(2):
                        ot = opool.tile([128, S], dtr, tag="ot", name="ot")
                        for tb in range(2):
                            e_half(op01[j], wt_o01[j], j, tb, [7],
                                   start=False, stop=True, ot=ot)
                    for oc in range(2, E // 128):
                        wt = wo_dma(oc)
                        op = psO.tile([128, S], dt, tag="op", name="op")
                        ot = opool.tile([128, S], dtr, tag="ot", name="ot")
                        last = oc == E // 128 - 1
                        for tb in range(2):
                            e_half(op, wt, oc, tb, list(range(HL)),
                                   start=True, stop=True,
                                   ot=None if (last and tb == 1) else ot)
                        if last:
                            for q in range(2):
                                lo = 512 + q * 256
                                nc.scalar.copy(ot[:, lo:lo + 256],
                                               op[:, lo:lo + 256])
                                nc.sync.dma_start(
                                    out=out_d[oc * 128:(oc + 1) * 128,
                                              lo:lo + 256],
                                    in_=ot[:, lo:lo + 256])

    nc.compile()
    return nc


def _rope_tables():
    inv = 1.0 / (10000.0 ** (np.arange(0, HD, 2, dtype=np.float32) / HD))  # [64]
    ang = np.arange(S, dtype=np.float32)[None, :] * inv[:, None]           # [64, S]
    cos = np.concatenate([np.cos(ang), np.cos(ang)], axis=0).astype(np.float32)   # [128, S]
    sin = np.sin(ang)
    sinp = np.concatenate([-sin, sin], axis=0).astype(np.float32)          # [128, S]
    return cos, sinp


def _rearrange_w(w, n_chunks):
    # [E_rows, n_chunks*128] -> [n_chunks*128, E_rows] blocks: row cc*128+p
    # holds w[c*128+p, cc*128+m] at col c*128+m
    e_rows = w.shape[0]
    c = e_rows // 128
    return np.ascontiguousarray(
        w.reshape(c, 128, n_chunks, 128).transpose(2, 1, 0, 3).reshape(
            n_chunks * 128, e_rows))


def make_in_maps(x, wq, wk, wv, wo):
    cos, sinp = _rope_tables()
    ndt = np.float16 if MM_DT == "float16" else np.float32
    x = np.ascontiguousarray(x, dtype=np.float32)
    in_maps = []
    for c in range(N_CORES):
        b, r = c // TP, c % TP
        in_maps.append({
            "xt": np.ascontiguousarray(x[b].T).astype(ndt),
            "wq": _rearrange_w(
                wq[:, r * QCOLS:(r + 1) * QCOLS].astype(ndt), HL),
            "wk": _rearrange_w(
                wk[:, r * KVCOLS:(r + 1) * KVCOLS].astype(ndt), KVL),
            "wv": _rearrange_w(
                wv[:, r * KVCOLS:(r + 1) * KVCOLS].astype(ndt), KVL),
            "wo": _rearrange_w(
                wo[r * QCOLS:(r + 1) * QCOLS, :].astype(ndt), ECH),
            "cos": cos.astype(ndt),
            "sinp": sinp.astype(ndt),
        })
    return in_maps


def kernel(x, wq, wk, wv, wo):
    global _PROGRAM
    from concourse.bass_utils import run_bass_kernel_spmd

    if _PROGRAM is None:
        _PROGRAM = _build_program()
    nc = _PROGRAM

    res = run_bass_kernel_spmd(nc, make_in_maps(x, wq, wk, wv, wo),
                               list(range(N_CORES)))

    out = np.zeros((B, S, E), dtype=np.float32)
    for c in range(N_CORES):
        b = c // TP
        out[b] += res.results[c]["out_t"].T.astype(np.float32)
    return out
